# revision 1
# baseline (speedup 1.0000x reference)
"""Depthwise 3x3 conv over each depth slice of x[B,H,W,D,C] on 8 trn2 cores.

Strategy:
  - Data-parallel over batch: core i handles x[i] ([H,W,D,C] = [64,64,32,64]).
  - Per core, loop over 16 depth-pair groups; partitions = (d_parity, C) = 128,
    free axis = spatial (H*W) so the per-(d,c) tap weights are per-partition
    scalars and each tap is one fused (x*w + acc) instruction.
  - HBM has C contiguous, so the (spatial, channel) <-> (channel, spatial)
    layout change is done on-chip with PE transposes (128x128 blocks).
  - SAME zero padding handled by a 65-stride padded slab with zeroed guard
    rows/pad column so every tap is a flat shifted read.
"""

import os
from contextlib import ExitStack

import numpy as np

import concourse.bass as bass
import concourse.mybir as mybir
import concourse.tile as tile
from concourse.bass_utils import run_bass_kernel_spmd
from concourse.masks import make_identity
from concourse.tile import add_dep_helper

F32 = mybir.dt.float32

B, H, W, D, C = 8, 64, 64, 32, 64
G = D // 2              # 16 depth-pair groups per core
RS = W + 1              # 65: padded row stride (col 64 of each row is zero)
DATA0 = RS + 1          # 66: flat offset of (h=0, w=0) in the slab
SLAB = DATA0 + 64 * RS + RS + 1   # 66 + 4160 + 66 = 4292
CONVL = 64 * RS         # 4160 = span of a [64 rows x 65] view

MULT = mybir.AluOpType.mult
ADD = mybir.AluOpType.add

# Tap split: first N_PE_TAPS run as diagonal matmuls on the TensorEngine
# accumulating into PSUM (plus the bias, seeded there too); the rest run as
# fused scalar_tensor_tensor accumulates on the vector engine, whose first
# op reads the PSUM partial as its accumulator input.
ALL_TAPS = [(dh, dw) for dh in (-1, 0, 1) for dw in (-1, 0, 1)]
# PE diag-matmul taps need float32r to stream at full rate, but the BIR
# verifier then requires every producer feeding the matmul to round to
# f32r (including the x slab itself) — unacceptable precision risk, so the
# conv runs entirely on the vector engine (N_PE_TAPS = 0).
N_PE_TAPS = 0
PE_TAPS = ALL_TAPS[:N_PE_TAPS]
DVE_TAPS = ALL_TAPS[N_PE_TAPS:]
# GPSIMD offload: walrus accepts tensor_scalar/tensor_tensor on Pool, so
# gpsimd builds a (bias + N_GPS_TAPS taps) partial that seeds the DVE chain.
# Products ~1-2 cyc/elem, adds ~2.6 cyc/elem on the 8 Q7 cores.
N_GPS_TAPS = 0
# run the 128x128 PE transposes with float32r operands (1.5 vs 2.0
# cycles/row). Identity-matmul data movement; exactness verified on HW.
TRANSPOSE_F32R = False


def _build_nc():
    nc = bass.Bass("TRN2", target_bir_lowering=False, debug=False)
    xs = nc.dram_tensor("xs", [H, W, D, C], F32, kind="ExternalInput").ap()
    ws = nc.dram_tensor("ws", [128, G * 9], F32, kind="ExternalInput").ap()
    bs = nc.dram_tensor("bs", [128, G], F32, kind="ExternalInput").ap()
    ys = nc.dram_tensor("ys", [H, W, D, C], F32, kind="ExternalOutput").ap()

    with tile.TileContext(nc) as tc, ExitStack() as ctx:
        consts = ctx.enter_context(tc.tile_pool(name="consts", bufs=1))
        ident = consts.tile([128, 128], F32)
        make_identity(nc, ident[:])
        ones = consts.tile([128, 512], F32)
        nc.vector.memset(ones[:], 1.0)
        wst = consts.tile([128, G * 9], F32)
        nc.sync.dma_start(wst[:], ws)
        bst = consts.tile([128, G], F32)
        nc.sync.dma_start(bst[:], bs)

        xdp = ctx.enter_context(tc.tile_pool(name="xd", bufs=3))
        xap = ctx.enter_context(tc.tile_pool(name="xa", bufs=3))
        yp = ctx.enter_context(tc.tile_pool(name="y", bufs=2))
        ydp = ctx.enter_context(tc.tile_pool(name="yd", bufs=2))
        dgp = ctx.enter_context(tc.tile_pool(name="diag", bufs=2))
        tp = ctx.enter_context(tc.tile_pool(name="gpspart", bufs=1))
        pin = ctx.enter_context(
            tc.tile_pool(name="pin", bufs=3, space=bass.MemorySpace.PSUM)
        )
        pout = ctx.enter_context(
            tc.tile_pool(name="pout", bufs=3, space=bass.MemorySpace.PSUM)
        )
        pp = ctx.enter_context(
            tc.tile_pool(name="pp", bufs=1, space=bass.MemorySpace.PSUM)
        )
        pdum = ctx.enter_context(
            tc.tile_pool(name="pdum", bufs=1, space=bass.MemorySpace.PSUM)
        )

        # PE instructions accept at most ONE sync wait in this toolchain, so:
        #  - an ACT "toucher" is made the first accessor of every psum tile
        #    (it can carry the multi-engine slot release-set),
        #  - tiny absorber matmuls into a write-only dummy psum tile observe
        #    one semaphore each (DMA / toucher / y2) before the real
        #    transposes, which are pinned behind them with add_dep_helper.
        dummy = pdum.tile([128, 8], F32)

        def pe_absorb(col, dep=None):
            mm = nc.tensor.matmul(
                dummy[0:1, 0:1], col, ident[:, 0:1], skip_group_check=True
            )
            if dep is not None:
                add_dep_helper(mm.ins, dep.ins, reason="observe tick")
            return mm

        pe_absorb(ident[:, 0:1])  # PE observes the identity build once

        pending_out = None
        for g in range(G):
            # ---- load: [128 spatial, 32 blocks, 128 ch] (512B bursts in HBM)
            src = xs[:, :, 2 * g : 2 * g + 2, :].rearrange(
                "(j ph) w dp c -> (ph w) j (dp c)", ph=2
            )
            xd = xdp.tile([128, 32, 128], F32, tag="xd")
            nc.sync.dma_start(xd[:], src)

            # ---- padded slab (channel-major); pads zeroed on gpsimd (idle)
            xa = xap.tile([128, SLAB], F32, tag="xa")
            nc.gpsimd.memset(xa[:, 0:DATA0], 0.0)
            nc.gpsimd.memset(xa[:, DATA0 + 63 * RS + 64 : SLAB], 0.0)
            padcol = xa[:, DATA0 + 64 : DATA0 + 64 + CONVL].rearrange(
                "p (r o) -> p r o", o=RS
            )[:, :, 0:1]
            nc.gpsimd.memset(padcol, 0.0)

            absA = pe_absorb(xd[:, 0, 0:1])  # PE observes xd's DMA
            last_copy = None
            for q in range(8):
                pt = pin.tile([128, 512], F32, tag="pin")
                touch = pt[0:1, :].rearrange("p (j c) -> p j c", j=4)[:, :, 0:1]
                tch = nc.scalar.copy(
                    touch, ident[0:1, 0:4].rearrange("p (j c) -> p j c", c=1)
                )
                absB = pe_absorb(ident[:, 0:1], dep=tch)
                for jo in range(4):
                    j = 4 * q + jo
                    if TRANSPOSE_F32R:
                        R = mybir.dt.float32r
                        t = nc.tensor.transpose(
                            pt[:, 128 * jo : 128 * (jo + 1)].bitcast(R),
                            xd[:, j, :].bitcast(R),
                            ident[:].bitcast(R),
                        )
                    else:
                        t = nc.tensor.transpose(
                            pt[:, 128 * jo : 128 * (jo + 1)], xd[:, j, :], ident[:]
                        )
                    add_dep_helper(t.ins, absB.ins, reason="after toucher-obs")
                    add_dep_helper(t.ins, absA.ins, reason="after dma-obs")
                dst = xa[:, DATA0 + 520 * q : DATA0 + 520 * q + 520].rearrange(
                    "p (j r b) -> p j r b", j=4, b=RS
                )[:, :, :, 0:64]
                srcp = pt[:].rearrange("p (j r b) -> p j r b", j=4, b=64)
                last_copy = nc.scalar.copy(dst, srcp)

            # ---- conv: y[h,w] = b + sum_t w_t * x[h+dh, w+dw]
            # PE: bias + N_PE_TAPS taps as diag-matmuls accumulating into a
            # PSUM quarter; DVE: remaining taps as fused STT, first one
            # reading the PSUM partial, last one writing y2.
            def wap(dh, dw, g=g):
                i = g * 9 + (dh + 1) * 3 + (dw + 1)
                return wst[:, i : i + 1]

            y = yp.tile([128, 4096], F32, tag="y")
            y2 = yp.tile([128, 4096], F32, tag="y2")

            if not PE_TAPS:
                yv = y[:].rearrange("p (a b) -> p a b", b=64)
                y2v = y2[:].rearrange("p (a b) -> p a b", b=64)

                def xsh(dh, dw, xa=xa):
                    s0 = DATA0 + dh * RS + dw
                    return xa[:, s0 : s0 + CONVL].rearrange(
                        "p (a b) -> p a b", b=RS
                    )[:, :, 0:64]

                gps_taps = DVE_TAPS[:N_GPS_TAPS]
                dve_taps = DVE_TAPS[N_GPS_TAPS:]
                if gps_taps:
                    # gpsimd partial: t1 = b + sum of gps taps, using y as
                    # scratch for the 2nd+ products (DVE overwrites y later).
                    t1 = tp.tile([128, 4096], F32, tag="t1")
                    t1v = t1[:].rearrange("p (a b) -> p a b", b=64)
                    (h0, w0) = gps_taps[0]
                    nc.gpsimd.tensor_scalar(
                        t1v, xsh(h0, w0), wap(h0, w0), bst[:, g : g + 1],
                        MULT, ADD,
                    )
                    for dh, dw in gps_taps[1:]:
                        nc.gpsimd.tensor_scalar(
                            yv, xsh(dh, dw), wap(dh, dw), None, MULT
                        )
                        nc.gpsimd.tensor_tensor(t1v, t1v, yv, ADD)
                    head = t1v
                    rest = dve_taps
                else:
                    # chain head computed on the scalar engine (it has
                    # slack): y = w*x + b via activation Identity with
                    # per-partition scale/bias — frees one DVE pass.
                    (sh, sw), rest = dve_taps[0], dve_taps[1:]
                    nc.scalar.activation(
                        yv,
                        xsh(sh, sw),
                        mybir.ActivationFunctionType.Identity,
                        bias=bst[:, g : g + 1],
                        scale=wap(sh, sw),
                    )
                    head = yv
                for i, (dh, dw) in enumerate(rest):
                    out = y2v if i == len(rest) - 1 else yv
                    in1 = head if i == 0 else yv
                    nc.vector.scalar_tensor_tensor(
                        out, xsh(dh, dw), wap(dh, dw), in1, MULT, ADD
                    )

            diag_b = dgp.tile([128, 128], F32, tag="dbias")
            nc.vector.tensor_scalar(
                diag_b[:], ident[:], bst[:, g : g + 1], None, MULT
            )
            diags = []
            last_diag = None
            for i, (dh, dw) in enumerate(PE_TAPS):
                dt_ = dgp.tile([128, 128], F32, tag=f"d{i}")
                last_diag = nc.vector.tensor_scalar(
                    dt_[:], ident[:], wap(dh, dw), None, MULT
                )
                diags.append(dt_)

            abs_xa = pe_absorb(ident[:, 0:1], dep=last_copy)
            abs_dg = pe_absorb(ident[:, 0:1], dep=last_diag)

            for q in range(4 if PE_TAPS else 0):
                Pq = pp.tile([128, 1024], F32, tag="pp")
                touch = Pq[0:1, :].rearrange("p (h c) -> p h c", h=2)[:, :, 0:1]
                tch = nc.scalar.copy(
                    touch, ident[0:1, 0:2].rearrange("p (h c) -> p h c", c=1)
                )
                absB = pe_absorb(ident[:, 0:1], dep=tch)
                # float32r: same bits as fp32, PE multiplies at reduced
                # precision but streams at 1 cycle/row instead of fp32's 4.
                F32R = mybir.dt.float32r
                for h in range(2):
                    r0 = 16 * q + 8 * h
                    mms = []
                    mm = nc.tensor.matmul(
                        Pq[:, 512 * h : 512 * (h + 1)],
                        diag_b[:].bitcast(F32R),
                        ones[:].bitcast(F32R),
                        start=True,
                        stop=False,
                    )
                    mms.append(mm)
                    for i, (dh, dw) in enumerate(PE_TAPS):
                        o = DATA0 + dh * RS + dw + r0 * RS
                        rhs = xa[:, o : o + 520].rearrange(
                            "p (r b) -> p r b", b=RS
                        )[:, :, 0:64]
                        mm = nc.tensor.matmul(
                            Pq[:, 512 * h : 512 * (h + 1)],
                            diags[i][:].bitcast(F32R),
                            rhs.bitcast(F32R),
                            start=False,
                            stop=(i == len(PE_TAPS) - 1),
                        )
                        mms.append(mm)
                    for mm in mms:
                        add_dep_helper(mm.ins, absB.ins, reason="after toucher")
                        add_dep_helper(mm.ins, abs_xa.ins, reason="after xa")
                        add_dep_helper(mm.ins, abs_dg.ins, reason="after diags")

                yq = y[:, 1024 * q : 1024 * (q + 1)].rearrange(
                    "p (a b) -> p a b", b=64
                )
                y2q = y2[:, 1024 * q : 1024 * (q + 1)].rearrange(
                    "p (a b) -> p a b", b=64
                )
                pv = Pq[:].rearrange("p (a b) -> p a b", b=64)
                for i, (dh, dw) in enumerate(DVE_TAPS):
                    o = DATA0 + dh * RS + dw + 16 * q * RS
                    in0 = xa[:, o : o + 1040].rearrange("p (a b) -> p a b", b=RS)[
                        :, :, 0:64
                    ]
                    in1 = pv if i == 0 else yq
                    out = y2q if i == len(DVE_TAPS) - 1 else yq
                    nc.vector.scalar_tensor_tensor(
                        out, in0, wap(dh, dw), in1, MULT, ADD
                    )

            # ---- transpose back + store, emitted one group LATE so this
            # group's in-transposes aren't stuck behind the previous group's
            # out-transposes in PE program order (PE is in-order; the
            # out-path is gated on the conv chain's end).
            def out_path(y2=y2, g=g):
                yd = ydp.tile([128, 32, 128], F32, tag="yd")
                absC = pe_absorb(y2[:, 0:1])  # PE observes y2's final writer
                for q in range(8):
                    pt = pout.tile([128, 512], F32, tag="pout")
                    touch = pt[0:1, :].rearrange("p (j c) -> p j c", j=4)[
                        :, :, 0:1
                    ]
                    tch = nc.scalar.copy(
                        touch, ident[0:1, 0:4].rearrange("p (j c) -> p j c", c=1)
                    )
                    absB = pe_absorb(ident[:, 0:1], dep=tch)
                    for jo in range(4):
                        j = 4 * q + jo
                        t = nc.tensor.transpose(
                            pt[:, 128 * jo : 128 * (jo + 1)],
                            y2[:, 128 * j : 128 * (j + 1)],
                            ident[:],
                        )
                        add_dep_helper(t.ins, absB.ins, reason="after toucher")
                        add_dep_helper(t.ins, absC.ins, reason="after y2-obs")
                    nc.scalar.copy(
                        yd[:, 4 * q : 4 * q + 4, :],
                        pt[:].rearrange("p (j c) -> p j c", j=4),
                    )
                dst = ys[:, :, 2 * g : 2 * g + 2, :].rearrange(
                    "(j ph) w dp c -> (ph w) j (dp c)", ph=2
                )
                nc.sync.dma_start(dst, yd[:])

            if pending_out is not None:
                pending_out()
            pending_out = out_path

        pending_out()

    return nc


# walrus setupSyncWait caps per engine struct: PE Matmult takes 1 sem wait,
# ACT/DVE/Pool compute ops take 2. Tile sometimes attaches more (psum slot
# release-sets). Hoist the excess onto injected same-engine Drains (Tile's
# own epilogue Drain carries 12 waits, so Drain accepts many).
_WAIT_CAPS = {"PE": 1, "Activation": 1, "DVE": 1, "Pool": 1, "SP": 1}
_SPLIT_SEQ = [0]


def _split_waits(nc):
    fn = nc.m.functions[0]
    nsplit = 0
    for blk in fn.blocks:
        out = []
        changed = False
        for ins in blk.instructions:
            si = ins.sync_info
            waits = list(si.on_wait) if si is not None and si.on_wait else []
            eng = getattr(ins, "engine", None)
            engname = getattr(eng, "value", None) or str(eng)
            cap = _WAIT_CAPS.get(engname)
            if cap is not None and len(waits) > cap:
                excess, keep = waits[:-cap], waits[-cap:]
                for w in excess:
                    _SPLIT_SEQ[0] += 1
                    d = mybir.InstDrain(name=f"I-ws{_SPLIT_SEQ[0]}", ins=[], outs=[])
                    d.engine = eng
                    d.sync_info = mybir.SyncInfo(on_wait=[w], on_update=[])
                    out.append(d)
                ins.sync_info = mybir.SyncInfo(
                    on_wait=keep, on_update=list(si.on_update or [])
                )
                changed = True
                nsplit += 1
            out.append(ins)
        if changed:
            blk.instructions = out
    return nsplit


_NC_CACHE = None


def _get_nc():
    global _NC_CACHE
    if _NC_CACHE is None:
        nc = _build_nc()
        _split_waits(nc)
        _NC_CACHE = nc
    return _NC_CACHE


class Runner:
    """Persistent PJRT executor for an SPMD bass module (axon path).

    Mirrors bass2jax.run_bass_via_pjrt's multi-core branch but keeps the
    jitted callable so repeated (timed) invocations don't recompile.
    """

    def __init__(self, nc, n_cores=8):
        import jax
        from jax.experimental.shard_map import shard_map
        from jax.sharding import Mesh, PartitionSpec
        from concourse import bass2jax

        bass2jax.install_neuronx_cc_hook()
        self.jax = jax
        self.nc = nc
        self.n = n_cores
        partition_name = (
            nc.partition_id_tensor.name if nc.partition_id_tensor else None
        )
        in_names, out_names, out_avals = [], [], []
        for alloc in nc.m.functions[0].allocations:
            if not isinstance(alloc, mybir.MemoryLocationSet):
                continue
            name = alloc.memorylocations[0].name
            if alloc.kind == "ExternalInput":
                if name != partition_name:
                    in_names.append(name)
            elif alloc.kind == "ExternalOutput":
                out_names.append(name)
                out_avals.append(
                    jax.core.ShapedArray(
                        tuple(alloc.tensor_shape), mybir.dt.np(alloc.dtype)
                    )
                )
        self.in_names = list(in_names)
        self.out_names = out_names
        self.out_avals = out_avals
        bind_in_names = list(in_names) + list(out_names)
        if partition_name is not None:
            bind_in_names.append(partition_name)
        bind_in_names = tuple(bind_in_names)
        n_params = len(in_names)
        n_outs = len(out_names)

        def _body(*args):
            operands = list(args)
            if partition_name is not None:
                operands.append(bass2jax.partition_id_tensor())
            outs = bass2jax._bass_exec_p.bind(
                *operands,
                out_avals=tuple(out_avals),
                in_names=bind_in_names,
                out_names=tuple(out_names),
                lowering_input_output_aliases=(),
                sim_require_finite=True,
                sim_require_nnan=True,
                nc=nc,
            )
            return tuple(outs)

        devices = jax.devices()[:n_cores]
        self.mesh = Mesh(np.asarray(devices), ("core",))
        self.spec = PartitionSpec("core")
        in_specs = (self.spec,) * (n_params + n_outs)
        out_specs = (self.spec,) * n_outs
        donate = tuple(range(n_params, n_params + n_outs))
        self.fn = jax.jit(
            shard_map(
                _body,
                mesh=self.mesh,
                in_specs=in_specs,
                out_specs=out_specs,
                check_rep=False,
            ),
            donate_argnums=donate,
            keep_unused=True,
        )
        sharding = jax.sharding.NamedSharding(self.mesh, self.spec)
        self.zeros_fn = jax.jit(
            lambda: tuple(
                self.jax.numpy.zeros((n_cores * a.shape[0], *a.shape[1:]), a.dtype)
                for a in out_avals
            ),
            out_shardings=(sharding,) * n_outs,
        )

    def put_inputs(self, in_maps):
        """in_maps: per-core dict name->np.ndarray. Returns device arrays."""
        jax = self.jax
        sharding = jax.sharding.NamedSharding(self.mesh, self.spec)
        arrs = []
        for name in self.in_names:
            cat = np.concatenate([np.asarray(m[name]) for m in in_maps], axis=0)
            arrs.append(jax.device_put(cat, sharding))
        jax.block_until_ready(arrs)
        return arrs

    def __call__(self, dev_inputs):
        zs = self.zeros_fn()
        self.jax.block_until_ready(zs)
        out = self.fn(*dev_inputs, *zs)
        self.jax.block_until_ready(out)
        return out

    def time_it(self, dev_inputs, reps=10):
        import time as _t

        ts = []
        for _ in range(reps):
            zs = self.zeros_fn()
            self.jax.block_until_ready(zs)
            t0 = _t.perf_counter()
            out = self.fn(*dev_inputs, *zs)
            self.jax.block_until_ready(out)
            ts.append(_t.perf_counter() - t0)
        return ts

    def to_numpy(self, out):
        n = self.n
        return [
            {
                name: np.asarray(out[i]).reshape(n, *self.out_avals[i].shape)[c]
                for i, name in enumerate(self.out_names)
            }
            for c in range(n)
        ]


_RUNNER = None


def _get_runner():
    global _RUNNER
    if _RUNNER is None:
        _RUNNER = Runner(_get_nc(), B)
    return _RUNNER


def _prep_wb(w, b):
    # ws[p, g*9 + kh*3 + kw] = w[2g + p//64, kh, kw, p%64]
    w = np.asarray(w, dtype=np.float32).reshape(G, 2, 9, C)  # (g, dp, tap, c)
    ws = np.ascontiguousarray(w.transpose(1, 3, 0, 2).reshape(128, G * 9))
    b = np.asarray(b, dtype=np.float32).reshape(G, 2, C)
    bs = np.ascontiguousarray(b.transpose(1, 2, 0).reshape(128, G))
    return ws, bs


def _in_maps(inputs):
    x = np.asarray(inputs["x"], dtype=np.float32)
    ws, bs = _prep_wb(inputs["w"], inputs["b"])
    return [{"xs": np.ascontiguousarray(x[i]), "ws": ws, "bs": bs} for i in range(B)]


def kernel(**inputs) -> np.ndarray:
    r = _get_runner()
    dev = r.put_inputs(_in_maps(inputs))
    res = r.to_numpy(r(dev))
    return np.stack([m["ys"] for m in res], axis=0)



# revision 13
# speedup vs baseline: 2.1160x; 2.1160x over previous
"""Depthwise 3x3 conv over each depth slice of x[B,H,W,D,C] on 8 trn2 cores.

Strategy (v2 — engine-balanced range split):
  - Data-parallel over batch: core i handles x[i] ([H,W,D,C] = [64,64,32,64]).
  - Per core, loop over 16 depth-pair groups; partitions = (d_parity, C) = 128,
    free axis = spatial (H*W) so per-(d,c) tap weights are per-partition
    scalars.
  - HBM has C contiguous, so the (spatial, channel) <-> (channel, spatial)
    layout change is done on-chip with PE transposes (128x128 blocks).
  - SAME zero padding handled by a 65-stride padded slab with zeroed guard
    rows/pad column so every tap is a flat shifted read.
  - The 64 spatial rows of each group are split across three engines:
      rows [0, PE_ROWS):            9 diag-matmuls (f32r) accumulating in
                                    PSUM; ACT copies out, folding the bias.
      rows [PE_ROWS, +DVE_ROWS):    ACT head (w*x+b) then 8 fused
                                    scalar_tensor_tensor taps on DVE.
      rows [.., 64):                ACT head then 8 taps on GPSIMD (Pool),
                                    either fused STT or TS-mult + TT-add.
  - Diagonal weight matrices for the PE taps are built in ONE DVE
    tensor_tensor with broadcast APs (ident x w).
"""

import os
from contextlib import ExitStack

import numpy as np

import concourse.bass as bass
import concourse.mybir as mybir
import concourse.tile as tile
from concourse.bass_utils import run_bass_kernel_spmd
from concourse.masks import make_identity
from concourse.tile import add_dep_helper

F32 = mybir.dt.float32
F32R = mybir.dt.float32r

B, H, W, D, C = 8, 64, 64, 32, 64
G = D // 2              # 16 depth-pair groups per core
RS = W + 1              # 65: padded row stride (col 64 of each row is zero)
DATA0 = RS + 1          # 66: flat offset of (h=0, w=0) in the slab
SLAB = DATA0 + 64 * RS + RS + 1   # 66 + 4160 + 66 = 4292
CONVL = 64 * RS         # 4160 = span of a [64 rows x 65] view

MULT = mybir.AluOpType.mult
ADD = mybir.AluOpType.add
IDENT_F = mybir.ActivationFunctionType.Identity

ALL_TAPS = [(dh, dw) for dh in (-1, 0, 1) for dw in (-1, 0, 1)]

# ---- tunables -----------------------------------------------------------
PE_ROWS = 38            # rows on the TensorEngine (diag-matmul taps)
DVE_ROWS = 22           # rows on the vector engine (STT chain)
POOL_ROWS = 64 - PE_ROWS - DVE_ROWS
POOL_STT = False        # walrus rejects STT on Pool -> TS-mult + TT-add
SLAB_DT = F32R          # slab/diag dtype fed to PE matmuls (1 cyc/row)
TRANS_F32R = True       # run the 128x128 transposes as f32r (1.5 vs 2 cyc)
Y2_DT = F32R            # conv result dtype (f32r out-transposes)
ABSORBERS = True        # PE observer matmuls (cheap; shields 1-wait cap)


def _pe_chunks():
    """Split PE_ROWS into psum-sized chunks (>=4 rows keeps f32r fast)."""
    out, r = [], 0
    while r < PE_ROWS:
        n = min(8, PE_ROWS - r)
        out.append((r, n))
        r += n
    assert all(n >= 4 for _, n in out)
    return out


def _build_nc():
    nc = bass.Bass("TRN2", target_bir_lowering=False, debug=False)
    xs = nc.dram_tensor("xs", [H, W, D, C], F32, kind="ExternalInput").ap()
    ws = nc.dram_tensor("ws", [128, G * 9], F32, kind="ExternalInput").ap()
    bs = nc.dram_tensor("bs", [128, G], F32, kind="ExternalInput").ap()
    ys = nc.dram_tensor("ys", [H, W, D, C], F32, kind="ExternalOutput").ap()

    with tile.TileContext(nc) as tc, ExitStack() as ctx:
        consts = ctx.enter_context(tc.tile_pool(name="consts", bufs=1))
        ident = consts.tile([128, 128], F32)
        make_identity(nc, ident[:])
        # f32r identity for the transposes: walrus wants every compute
        # producer feeding an f32r matmul to declare an f32r-rounded output,
        # so materialize it via an ACT copy (f32r out is legal on ACT).
        identr = consts.tile([128, 128], F32R)
        nc.scalar.copy(identr[:], ident[:])
        wst = consts.tile([128, G * 9], F32)
        nc.sync.dma_start(wst[:], ws)
        bst = consts.tile([128, G], F32)
        nc.sync.dma_start(bst[:], bs)

        xdp = ctx.enter_context(tc.tile_pool(name="xd", bufs=3))
        xap = ctx.enter_context(tc.tile_pool(name="xa", bufs=2))
        dgp = ctx.enter_context(tc.tile_pool(name="diag", bufs=2))
        scp = ctx.enter_context(tc.tile_pool(name="scr", bufs=2))
        y2p = ctx.enter_context(tc.tile_pool(name="y2", bufs=2))
        ydp = ctx.enter_context(tc.tile_pool(name="yd", bufs=2))
        pin = ctx.enter_context(
            tc.tile_pool(name="pin", bufs=3, space=bass.MemorySpace.PSUM)
        )
        pcv = ctx.enter_context(
            tc.tile_pool(name="pcv", bufs=2, space=bass.MemorySpace.PSUM)
        )
        pout = ctx.enter_context(
            tc.tile_pool(name="pout", bufs=2, space=bass.MemorySpace.PSUM)
        )
        pdum = ctx.enter_context(
            tc.tile_pool(name="pdum", bufs=1, space=bass.MemorySpace.PSUM)
        )

        dummy = pdum.tile([128, 8], F32)

        def pe_absorb(col, dep=None):
            if not ABSORBERS:
                return None
            if col.dtype is not F32:
                col = col.bitcast(F32)
            mm = nc.tensor.matmul(
                dummy[0:1, 0:1], col, ident[:, 0:1], skip_group_check=True
            )
            if dep is not None:
                add_dep_helper(mm.ins, dep.ins, reason="observe")
            return mm

        def pin_dep(mm, *deps):
            for d in deps:
                if d is not None:
                    add_dep_helper(mm.ins, d.ins, reason="after-observer")

        pe_absorb(ident[:, 0:1])  # PE observes the identity build once

        pending_out = None
        for g in range(G):
            # ---- load: [128 spatial, 32 blocks, 128 ch] (512B bursts)
            src = xs[:, :, 2 * g : 2 * g + 2, :].rearrange(
                "(j ph) w dp c -> (ph w) j (dp c)", ph=2
            )
            # xd is f32r-typed so the DMA itself is the declared f32r
            # producer for the f32r in-transposes (same bits as f32)
            xd = xdp.tile([128, 32, 128], F32R if TRANS_F32R else F32, tag="xd")
            nc.sync.dma_start(xd[:], src if not TRANS_F32R else src.bitcast(F32R))

            # ---- padded slab (channel-major); pads zeroed on gpsimd
            xa = xap.tile([128, SLAB], SLAB_DT, tag="xa")
            xaf = xa[:] if SLAB_DT is F32 else xa[:].bitcast(F32)
            nc.gpsimd.memset(xaf[:, 0:DATA0], 0.0)
            nc.gpsimd.memset(xaf[:, DATA0 + 63 * RS + 64 : SLAB], 0.0)
            padcol = xaf[:, DATA0 + 64 : DATA0 + 64 + CONVL].rearrange(
                "p (r o) -> p r o", o=RS
            )[:, :, 0:1]
            pad_last = nc.gpsimd.memset(padcol, 0.0)

            absA = pe_absorb(xd[:, 0, 0:1])  # PE observes xd's DMA
            last_copy = None
            for q in range(8):
                pt = pin.tile([128, 512], F32, tag="pin")
                for jo in range(4):
                    j = 4 * q + jo
                    if TRANS_F32R:
                        t = nc.tensor.transpose(
                            pt[:, 128 * jo : 128 * (jo + 1)].bitcast(F32R),
                            xd[:, j, :],
                            identr[:],
                        )
                    else:
                        t = nc.tensor.transpose(
                            pt[:, 128 * jo : 128 * (jo + 1)], xd[:, j, :], ident[:]
                        )
                    pin_dep(t, absA)
                dst = xa[:, DATA0 + 520 * q : DATA0 + 520 * q + 520].rearrange(
                    "p (j r b) -> p j r b", j=4, b=RS
                )[:, :, :, 0:64]
                srcp = pt[:].rearrange("p (j r b) -> p j r b", j=4, b=64)
                last_copy = nc.scalar.copy(dst, srcp)

            # ---- shifted slab views -----------------------------------
            def xsh(dh, dw, r0, nr, xa=xa):
                s0 = DATA0 + dh * RS + dw + r0 * RS
                v = xa[:, s0 : s0 + nr * RS]
                if SLAB_DT is not F32:
                    v = v.bitcast(F32)
                return v.rearrange("p (r b) -> p r b", b=RS)[:, :, 0:64]

            def xshr(dh, dw, r0, nr, xa=xa):
                s0 = DATA0 + dh * RS + dw + r0 * RS
                return xa[:, s0 : s0 + nr * RS].rearrange(
                    "p (r b) -> p r b", b=RS
                )[:, :, 0:64]

            def wap(dh, dw, g=g):
                i = g * 9 + (dh + 1) * 3 + (dw + 1)
                return wst[:, i : i + 1]

            bias = bst[:, g : g + 1]
            y2 = y2p.tile([128, 4096], Y2_DT, tag="y2")

            def yv(r0, nr, y2=y2):
                return y2[:, r0 * 64 : (r0 + nr) * 64].rearrange(
                    "p (r w) -> p r w", w=64
                )

            # ---- diag build: ONE broadcast tensor_tensor on DVE
            diag = dgp.tile([128, 9 * 128], SLAB_DT, tag="diag")
            identb = ident[:].unsqueeze(1).broadcast_to([128, 9, 128])
            wb = (
                wst[:, g * 9 : g * 9 + 9]
                .unsqueeze(2)
                .broadcast_to([128, 9, 128])
            )
            # out dtype stays F32R: walrus requires producers feeding an
            # f32r matmul to declare f32r-rounded output
            dgv = diag[:].rearrange("p (t c) -> p t c", t=9)
            diag_done = nc.vector.tensor_tensor(dgv, identb, wb, MULT)

            # ---- ACT heads for the DVE and Pool ranges
            r_dve = PE_ROWS
            r_pool = PE_ROWS + DVE_ROWS
            (h0, w0), rest = ALL_TAPS[0], ALL_TAPS[1:]
            sc_d = scp.tile([128, DVE_ROWS * 64], F32, tag="scd")
            sc_dv = sc_d[:].rearrange("p (r w) -> p r w", w=64)
            nc.scalar.activation(
                sc_dv, xsh(h0, w0, r_dve, DVE_ROWS), IDENT_F,
                bias=bias, scale=wap(h0, w0),
            )
            sc_p = scp.tile([128, POOL_ROWS * 64], F32, tag="scp")
            sc_pv = sc_p[:].rearrange("p (r w) -> p r w", w=64)
            nc.scalar.activation(
                sc_pv, xsh(h0, w0, r_pool, POOL_ROWS), IDENT_F,
                bias=bias, scale=wap(h0, w0),
            )

            # ---- DVE range: 8 accumulating STT taps
            dve_last = None
            for i, (dh, dw) in enumerate(rest):
                out = yv(r_dve, DVE_ROWS) if i == len(rest) - 1 else sc_dv
                dve_last = nc.vector.scalar_tensor_tensor(
                    out, xsh(dh, dw, r_dve, DVE_ROWS), wap(dh, dw),
                    sc_dv, MULT, ADD,
                )

            # ---- Pool range: 8 taps on gpsimd
            pool_last = None
            if POOL_STT:
                for i, (dh, dw) in enumerate(rest):
                    out = yv(r_pool, POOL_ROWS) if i == len(rest) - 1 else sc_pv
                    pool_last = nc.gpsimd.scalar_tensor_tensor(
                        out, xsh(dh, dw, r_pool, POOL_ROWS), wap(dh, dw),
                        sc_pv, MULT, ADD,
                    )
            else:
                sc_q = scp.tile([128, POOL_ROWS * 64], F32, tag="scq")
                sc_qv = sc_q[:].rearrange("p (r w) -> p r w", w=64)
                for i, (dh, dw) in enumerate(rest):
                    out = yv(r_pool, POOL_ROWS) if i == len(rest) - 1 else sc_pv
                    nc.gpsimd.tensor_scalar(
                        sc_qv, xsh(dh, dw, r_pool, POOL_ROWS), wap(dh, dw),
                        None, MULT,
                    )
                    pool_last = nc.gpsimd.tensor_tensor(out, sc_pv, sc_qv, ADD)

            # ---- out-path of the PREVIOUS group (keeps PE fed while ACT
            # assembles this group's slab / psum copies)
            if pending_out is not None:
                pending_out()
                pending_out = None

            # ---- PE range: 9 diag-matmul taps per chunk, ACT copy+bias out
            abs_xa = pe_absorb(ident[:, 0:1], dep=last_copy)
            abs_pd = pe_absorb(ident[:, 0:1], dep=pad_last)
            abs_dg = pe_absorb(ident[:, 0:1], dep=diag_done)
            pe_copy_last = None
            for r0, nr in _pe_chunks():
                Pq = pcv.tile([128, nr * 64], F32, tag="pcv")
                for i, (dh, dw) in enumerate(ALL_TAPS):
                    if SLAB_DT is F32:
                        lhs = diag[:, 128 * i : 128 * (i + 1)].bitcast(F32R)
                        rhs = xshr(dh, dw, r0, nr).bitcast(F32R)
                    else:
                        lhs = diag[:, 128 * i : 128 * (i + 1)]
                        rhs = xshr(dh, dw, r0, nr)
                    mm = nc.tensor.matmul(
                        Pq[:], lhs, rhs,
                        start=(i == 0), stop=(i == len(ALL_TAPS) - 1),
                    )
                    pin_dep(mm, abs_xa, abs_pd, abs_dg)
                pe_copy_last = nc.scalar.activation(
                    yv(r0, nr),
                    Pq[:].rearrange("p (r w) -> p r w", w=64),
                    IDENT_F, bias=bias,
                )

            # ---- transpose back + store, one group late
            def out_path(y2=y2, g=g, dve_last=dve_last, pool_last=pool_last,
                         pe_copy_last=pe_copy_last):
                yd = ydp.tile([128, 32, 128], F32, tag="yd")
                a1 = pe_absorb(ident[:, 0:1], dep=dve_last)
                a2 = pe_absorb(ident[:, 0:1], dep=pool_last)
                a3 = pe_absorb(ident[:, 0:1], dep=pe_copy_last)
                for q in range(8):
                    pt = pout.tile([128, 512], F32, tag="pout")
                    for jo in range(4):
                        j = 4 * q + jo
                        yblk = y2[:, 128 * j : 128 * (j + 1)]
                        if Y2_DT is F32R:
                            t = nc.tensor.transpose(
                                pt[:, 128 * jo : 128 * (jo + 1)].bitcast(F32R),
                                yblk, identr[:],
                            )
                        else:
                            t = nc.tensor.transpose(
                                pt[:, 128 * jo : 128 * (jo + 1)], yblk, ident[:]
                            )
                        pin_dep(t, a1, a2, a3)
                    nc.scalar.copy(
                        yd[:, 4 * q : 4 * q + 4, :],
                        pt[:].rearrange("p (j c) -> p j c", j=4),
                    )
                dst = ys[:, :, 2 * g : 2 * g + 2, :].rearrange(
                    "(j ph) w dp c -> (ph w) j (dp c)", ph=2
                )
                nc.sync.dma_start(dst, yd[:])

            pending_out = out_path

        pending_out()

    return nc


# walrus setupSyncWait caps per engine struct: PE Matmult takes 1 sem wait,
# ACT/DVE/Pool compute ops take 2. Tile sometimes attaches more (psum slot
# release-sets). Hoist the excess onto injected same-engine Drains (Tile's
# own epilogue Drain carries 12 waits, so Drain accepts many).
_WAIT_CAPS = {"PE": 1, "Activation": 1, "DVE": 1, "Pool": 1, "SP": 1}
_SPLIT_SEQ = [0]


def _split_waits(nc):
    fn = nc.m.functions[0]
    nsplit = 0
    for blk in fn.blocks:
        out = []
        changed = False
        for ins in blk.instructions:
            si = ins.sync_info
            waits = list(si.on_wait) if si is not None and si.on_wait else []
            eng = getattr(ins, "engine", None)
            engname = getattr(eng, "value", None) or str(eng)
            cap = _WAIT_CAPS.get(engname)
            if cap is not None and len(waits) > cap:
                excess, keep = waits[:-cap], waits[-cap:]
                for w in excess:
                    _SPLIT_SEQ[0] += 1
                    d = mybir.InstDrain(name=f"I-ws{_SPLIT_SEQ[0]}", ins=[], outs=[])
                    d.engine = eng
                    d.sync_info = mybir.SyncInfo(on_wait=[w], on_update=[])
                    out.append(d)
                ins.sync_info = mybir.SyncInfo(
                    on_wait=keep, on_update=list(si.on_update or [])
                )
                changed = True
                nsplit += 1
            out.append(ins)
        if changed:
            blk.instructions = out
    return nsplit


_NC_CACHE = None


def _get_nc():
    global _NC_CACHE
    if _NC_CACHE is None:
        nc = _build_nc()
        _split_waits(nc)
        _NC_CACHE = nc
    return _NC_CACHE


class Runner:
    """Persistent PJRT executor for an SPMD bass module (axon path).

    Mirrors bass2jax.run_bass_via_pjrt's multi-core branch but keeps the
    jitted callable so repeated (timed) invocations don't recompile.
    """

    def __init__(self, nc, n_cores=8):
        import jax
        from jax.experimental.shard_map import shard_map
        from jax.sharding import Mesh, PartitionSpec
        from concourse import bass2jax

        bass2jax.install_neuronx_cc_hook()
        self.jax = jax
        self.nc = nc
        self.n = n_cores
        partition_name = (
            nc.partition_id_tensor.name if nc.partition_id_tensor else None
        )
        in_names, out_names, out_avals = [], [], []
        for alloc in nc.m.functions[0].allocations:
            if not isinstance(alloc, mybir.MemoryLocationSet):
                continue
            name = alloc.memorylocations[0].name
            if alloc.kind == "ExternalInput":
                if name != partition_name:
                    in_names.append(name)
            elif alloc.kind == "ExternalOutput":
                out_names.append(name)
                out_avals.append(
                    jax.core.ShapedArray(
                        tuple(alloc.tensor_shape), mybir.dt.np(alloc.dtype)
                    )
                )
        self.in_names = list(in_names)
        self.out_names = out_names
        self.out_avals = out_avals
        bind_in_names = list(in_names) + list(out_names)
        if partition_name is not None:
            bind_in_names.append(partition_name)
        bind_in_names = tuple(bind_in_names)
        n_params = len(in_names)
        n_outs = len(out_names)

        def _body(*args):
            operands = list(args)
            if partition_name is not None:
                operands.append(bass2jax.partition_id_tensor())
            outs = bass2jax._bass_exec_p.bind(
                *operands,
                out_avals=tuple(out_avals),
                in_names=bind_in_names,
                out_names=tuple(out_names),
                lowering_input_output_aliases=(),
                sim_require_finite=True,
                sim_require_nnan=True,
                nc=nc,
            )
            return tuple(outs)

        devices = jax.devices()[:n_cores]
        self.mesh = Mesh(np.asarray(devices), ("core",))
        self.spec = PartitionSpec("core")
        in_specs = (self.spec,) * (n_params + n_outs)
        out_specs = (self.spec,) * n_outs
        donate = tuple(range(n_params, n_params + n_outs))
        self.fn = jax.jit(
            shard_map(
                _body,
                mesh=self.mesh,
                in_specs=in_specs,
                out_specs=out_specs,
                check_rep=False,
            ),
            donate_argnums=donate,
            keep_unused=True,
        )
        sharding = jax.sharding.NamedSharding(self.mesh, self.spec)
        self.zeros_fn = jax.jit(
            lambda: tuple(
                self.jax.numpy.zeros((n_cores * a.shape[0], *a.shape[1:]), a.dtype)
                for a in out_avals
            ),
            out_shardings=(sharding,) * n_outs,
        )

    def put_inputs(self, in_maps):
        """in_maps: per-core dict name->np.ndarray. Returns device arrays."""
        jax = self.jax
        sharding = jax.sharding.NamedSharding(self.mesh, self.spec)
        arrs = []
        for name in self.in_names:
            cat = np.concatenate([np.asarray(m[name]) for m in in_maps], axis=0)
            arrs.append(jax.device_put(cat, sharding))
        jax.block_until_ready(arrs)
        return arrs

    def __call__(self, dev_inputs):
        zs = self.zeros_fn()
        self.jax.block_until_ready(zs)
        out = self.fn(*dev_inputs, *zs)
        self.jax.block_until_ready(out)
        return out

    def time_it(self, dev_inputs, reps=10):
        import time as _t

        ts = []
        for _ in range(reps):
            zs = self.zeros_fn()
            self.jax.block_until_ready(zs)
            t0 = _t.perf_counter()
            out = self.fn(*dev_inputs, *zs)
            self.jax.block_until_ready(out)
            ts.append(_t.perf_counter() - t0)
        return ts

    def to_numpy(self, out):
        n = self.n
        return [
            {
                name: np.asarray(out[i]).reshape(n, *self.out_avals[i].shape)[c]
                for i, name in enumerate(self.out_names)
            }
            for c in range(n)
        ]


_RUNNER = None


def _get_runner():
    global _RUNNER
    if _RUNNER is None:
        _RUNNER = Runner(_get_nc(), B)
    return _RUNNER


def _prep_wb(w, b):
    # ws[p, g*9 + kh*3 + kw] = w[2g + p//64, kh, kw, p%64]
    w = np.asarray(w, dtype=np.float32).reshape(G, 2, 9, C)  # (g, dp, tap, c)
    ws = np.ascontiguousarray(w.transpose(1, 3, 0, 2).reshape(128, G * 9))
    b = np.asarray(b, dtype=np.float32).reshape(G, 2, C)
    bs = np.ascontiguousarray(b.transpose(1, 2, 0).reshape(128, G))
    return ws, bs


def _in_maps(inputs):
    x = np.asarray(inputs["x"], dtype=np.float32)
    ws, bs = _prep_wb(inputs["w"], inputs["b"])
    return [{"xs": np.ascontiguousarray(x[i]), "ws": ws, "bs": bs} for i in range(B)]


def kernel(**inputs) -> np.ndarray:
    r = _get_runner()
    dev = r.put_inputs(_in_maps(inputs))
    res = r.to_numpy(r(dev))
    return np.stack([m["ys"] for m in res], axis=0)


# revision 30
# speedup vs baseline: 2.3769x; 1.1233x over previous
"""Depthwise 3x3 conv over each depth slice of x[B,H,W,D,C] on 8 trn2 cores.

Strategy (v2 — engine-balanced range split):
  - Data-parallel over batch: core i handles x[i] ([H,W,D,C] = [64,64,32,64]).
  - Per core, loop over 16 depth-pair groups; partitions = (d_parity, C) = 128,
    free axis = spatial (H*W) so per-(d,c) tap weights are per-partition
    scalars.
  - HBM has C contiguous, so the (spatial, channel) <-> (channel, spatial)
    layout change is done on-chip with PE transposes (128x128 blocks).
  - SAME zero padding handled by a 65-stride padded slab with zeroed guard
    rows/pad column so every tap is a flat shifted read.
  - The 64 spatial rows of each group are split across three engines:
      rows [0, PE_ROWS):            9 diag-matmuls (f32r) accumulating in
                                    PSUM; ACT copies out, folding the bias.
      rows [PE_ROWS, +DVE_ROWS):    ACT head (w*x+b) then 8 fused
                                    scalar_tensor_tensor taps on DVE.
      rows [.., 64):                ACT head then 8 taps on GPSIMD (Pool),
                                    either fused STT or TS-mult + TT-add.
  - Diagonal weight matrices for the PE taps are built in ONE DVE
    tensor_tensor with broadcast APs (ident x w).
"""

import os
from contextlib import ExitStack

import numpy as np

import concourse.bass as bass
import concourse.mybir as mybir
import concourse.tile as tile
from concourse.bass_utils import run_bass_kernel_spmd
from concourse.masks import make_identity
from concourse.tile import add_dep_helper

F32 = mybir.dt.float32
F32R = mybir.dt.float32r
BF16 = mybir.dt.bfloat16

B, H, W, D, C = 8, 64, 64, 32, 64
G = D // 2              # 16 depth-pair groups per core
RS = W + 1              # 65: padded row stride (col 64 of each row is zero)
DATA0 = RS + 1          # 66: flat offset of (h=0, w=0) in the slab
SLAB = DATA0 + 64 * RS + RS + 1   # 66 + 4160 + 66 = 4292
CONVL = 64 * RS         # 4160 = span of a [64 rows x 65] view

MULT = mybir.AluOpType.mult
ADD = mybir.AluOpType.add
IDENT_F = mybir.ActivationFunctionType.Identity

ALL_TAPS = [(dh, dw) for dh in (-1, 0, 1) for dw in (-1, 0, 1)]

# ---- tunables -----------------------------------------------------------
PE_ROWS = 38            # rows on the TensorEngine (diag-matmul taps)
DVE_ROWS = 20           # rows on the vector engine (STT chain)
POOL_ROWS = 64 - PE_ROWS - DVE_ROWS
POOL_STT = False        # walrus rejects STT on Pool -> TS-mult + TT-add
SLAB_DT = F32R          # slab/diag dtype fed to PE matmuls (1 cyc/row)
TRANS_F32R = True       # run the 128x128 transposes as f32r (1.5 vs 2 cyc)
Y2_DT = BF16            # conv result dtype (bf16 = 1cyc/row out-transposes)
ABSORBERS = True        # PE observer matmuls (cheap; shields 1-wait cap)
DIAG_ON_POOL = True     # diag built on gpsimd (DVE fallback)
TRANS_LAG = 0           # iterations between dma issue and transposes
CONV_LAG = 1            # iterations between dma issue and conv
OUT_OFF = 1             # groups between conv and its out-path
PIN_W = 4               # transposes per in-psum tile (4 or 8)
XD_BUFS = 3
XA_BUFS = 3
SC_BUFS = 2
Y2_BUFS = 2
YD_BUFS = 2
PIN_BUFS = 3
PCV_BUFS = 2
POUT_BUFS = 2


def _pe_chunks():
    """Split PE_ROWS into psum-sized chunks (>=4 rows keeps f32r fast)."""
    out, r = [], 0
    while r < PE_ROWS:
        n = min(8, PE_ROWS - r)
        out.append((r, n))
        r += n
    assert all(n >= 4 for _, n in out)
    return out


def _build_nc():
    nc = bass.Bass("TRN2", target_bir_lowering=False, debug=False)
    xs = nc.dram_tensor("xs", [H, W, D, C], F32, kind="ExternalInput").ap()
    ws = nc.dram_tensor("ws", [128, G * 9], F32, kind="ExternalInput").ap()
    bs = nc.dram_tensor("bs", [128, G], F32, kind="ExternalInput").ap()
    ys = nc.dram_tensor("ys", [H, W, D, C], F32, kind="ExternalOutput").ap()

    with tile.TileContext(nc) as tc, ExitStack() as ctx:
        consts = ctx.enter_context(tc.tile_pool(name="consts", bufs=1))
        ident = consts.tile([128, 128], F32)
        make_identity(nc, ident[:])
        # f32r identity for the transposes: walrus wants every compute
        # producer feeding an f32r matmul to declare an f32r-rounded output,
        # so materialize it via an ACT copy (f32r out is legal on ACT).
        identr = consts.tile([128, 128], F32R)
        nc.scalar.copy(identr[:], ident[:])
        identb16 = consts.tile([128, 128], BF16)
        nc.scalar.copy(identb16[:], ident[:])
        wst = consts.tile([128, G * 9], F32)
        nc.sync.dma_start(wst[:], ws)
        bst = consts.tile([128, G], F32)
        nc.sync.dma_start(bst[:], bs)

        xdp = ctx.enter_context(tc.tile_pool(name="xd", bufs=XD_BUFS))
        xap = ctx.enter_context(tc.tile_pool(name="xa", bufs=XA_BUFS))
        dgp = ctx.enter_context(tc.tile_pool(name="diag", bufs=3))
        scp = ctx.enter_context(tc.tile_pool(name="scr", bufs=SC_BUFS))
        y2p = ctx.enter_context(tc.tile_pool(name="y2", bufs=Y2_BUFS))
        ydp = ctx.enter_context(tc.tile_pool(name="yd", bufs=YD_BUFS))
        pin = ctx.enter_context(
            tc.tile_pool(name="pin", bufs=PIN_BUFS, space=bass.MemorySpace.PSUM)
        )
        pcv = ctx.enter_context(
            tc.tile_pool(name="pcv", bufs=PCV_BUFS, space=bass.MemorySpace.PSUM)
        )
        pout = ctx.enter_context(
            tc.tile_pool(name="pout", bufs=POUT_BUFS, space=bass.MemorySpace.PSUM)
        )
        dummy = None
        if ABSORBERS:
            pdum = ctx.enter_context(
                tc.tile_pool(name="pdum", bufs=1, space=bass.MemorySpace.PSUM)
            )
            dummy = pdum.tile([128, 8], F32)

        def pe_absorb(col, dep=None):
            if not ABSORBERS:
                return None
            if col.dtype is not F32:
                col = col.bitcast(F32)
            mm = nc.tensor.matmul(
                dummy[0:1, 0:1], col, ident[:, 0:1], skip_group_check=True
            )
            if dep is not None:
                add_dep_helper(mm.ins, dep.ins, reason="observe")
            return mm

        def pin_dep(mm, *deps):
            for d in deps:
                if d is not None:
                    add_dep_helper(mm.ins, d.ins, reason="after-observer")

        pe_absorb(ident[:, 0:1])  # PE observes the identity build once

        # ---- software pipeline: in-path runs TWO groups ahead of the
        # conv so the PE matmuls never wait on the slab handoff (ACT
        # copies) inside PE's in-order stream, and the tap chains start
        # each cycle with their slab already resident.
        in_state = {}
        conv_state = {}

        def dma_issue(g):
            src = xs[:, :, 2 * g : 2 * g + 2, :].rearrange(
                "(j ph) w dp c -> (ph w) j (dp c)", ph=2
            )
            # xd is f32r-typed so the DMA itself is the declared f32r
            # producer for the f32r in-transposes (same bits as f32)
            xd = xdp.tile([128, 32, 128], F32R if TRANS_F32R else F32, tag="xd")
            nc.sync.dma_start(xd[:], src if not TRANS_F32R else src.bitcast(F32R))
            in_state[g] = dict(xd=xd)

        def trans_copy(g):
            xd = in_state[g]["xd"]
            xa = xap.tile([128, SLAB], SLAB_DT, tag="xa")
            xaf = xa[:] if SLAB_DT is F32 else xa[:].bitcast(F32)
            nc.gpsimd.memset(xaf[:, 0:DATA0], 0.0)
            nc.gpsimd.memset(xaf[:, DATA0 + 63 * RS + 64 : SLAB], 0.0)
            padcol = xaf[:, DATA0 + 64 : DATA0 + 64 + CONVL].rearrange(
                "p (r o) -> p r o", o=RS
            )[:, :, 0:1]
            pad_last = nc.gpsimd.memset(padcol, 0.0)

            absA = pe_absorb(xd[:, 0, 0:1])  # PE observes xd's DMA
            last_copy = None
            # tail (DVE/Pool range) chunks first so their chains can start
            nin = 32 // PIN_W
            order = list(range(nin // 2, nin)) + list(range(nin // 2))
            for k in order:
                pt = pin.tile([128, 128 * PIN_W], F32, tag="pin")
                for jo in range(PIN_W):
                    j = PIN_W * k + jo
                    if TRANS_F32R:
                        t = nc.tensor.transpose(
                            pt[:, 128 * jo : 128 * (jo + 1)].bitcast(F32R),
                            xd[:, j, :],
                            identr[:],
                        )
                    else:
                        t = nc.tensor.transpose(
                            pt[:, 128 * jo : 128 * (jo + 1)], xd[:, j, :], ident[:]
                        )
                    pin_dep(t, absA)
                span = 130 * PIN_W
                dst = xa[:, DATA0 + span * k : DATA0 + span * k + span].rearrange(
                    "p (j r b) -> p j r b", j=PIN_W, b=RS
                )[:, :, :, 0:64]
                srcp = pt[:].rearrange("p (j r b) -> p j r b", j=PIN_W, b=64)
                last_copy = nc.scalar.copy(dst, srcp)

            # diag for the PE taps: ONE broadcast tensor_tensor (on Pool,
            # which has slack; f32r out keeps walrus's producer rule happy)
            diag = dgp.tile([128, 9 * 128], SLAB_DT, tag="diag")
            identb = ident[:].unsqueeze(1).broadcast_to([128, 9, 128])
            wb = (
                wst[:, g * 9 : g * 9 + 9]
                .unsqueeze(2)
                .broadcast_to([128, 9, 128])
            )
            dgv = diag[:].rearrange("p (t c) -> p t c", t=9)
            if DIAG_ON_POOL:
                diag_done = nc.gpsimd.tensor_tensor(dgv, identb, wb, MULT)
            else:
                diag_done = nc.vector.tensor_tensor(dgv, identb, wb, MULT)

            in_state[g].update(
                xa=xa, diag=diag, last_copy=last_copy, pad_last=pad_last,
                diag_done=diag_done,
            )

        def xsh(g, dh, dw, r0, nr, dt=True):
            xa = in_state[g]["xa"]
            s0 = DATA0 + dh * RS + dw + r0 * RS
            v = xa[:, s0 : s0 + nr * RS]
            if dt and SLAB_DT is not F32:
                v = v.bitcast(F32)
            return v.rearrange("p (r b) -> p r b", b=RS)[:, :, 0:64]

        def wap(g, dh, dw):
            i = g * 9 + (dh + 1) * 3 + (dw + 1)
            return wst[:, i : i + 1]

        def chains(g):
            bias = bst[:, g : g + 1]
            y2 = y2p.tile([128, 4096], Y2_DT, tag="y2")
            conv_state[g] = dict(y2=y2)

            def yv(r0, nr):
                return y2[:, r0 * 64 : (r0 + nr) * 64].rearrange(
                    "p (r w) -> p r w", w=64
                )

            r_dve = PE_ROWS
            r_pool = PE_ROWS + DVE_ROWS
            (h0, w0), rest = ALL_TAPS[0], ALL_TAPS[1:]
            sc_d = scp.tile([128, DVE_ROWS * 64], F32, tag="scd")
            sc_dv = sc_d[:].rearrange("p (r w) -> p r w", w=64)
            nc.vector.tensor_scalar(
                sc_dv, xsh(g, h0, w0, r_dve, DVE_ROWS), wap(g, h0, w0), bias,
                MULT, ADD,
            )
            dve_last = None
            for i, (dh, dw) in enumerate(rest):
                out = yv(r_dve, DVE_ROWS) if i == len(rest) - 1 else sc_dv
                dve_last = nc.vector.scalar_tensor_tensor(
                    out, xsh(g, dh, dw, r_dve, DVE_ROWS), wap(g, dh, dw),
                    sc_dv, MULT, ADD,
                )

            pool_last = None
            if POOL_ROWS:
                sc_p = scp.tile([128, POOL_ROWS * 64], F32, tag="scp")
                sc_pv = sc_p[:].rearrange("p (r w) -> p r w", w=64)
                nc.gpsimd.tensor_scalar(
                    sc_pv, xsh(g, h0, w0, r_pool, POOL_ROWS), wap(g, h0, w0),
                    bias, MULT, ADD,
                )
                if POOL_STT:
                    for i, (dh, dw) in enumerate(rest):
                        out = (
                            yv(r_pool, POOL_ROWS)
                            if i == len(rest) - 1 else sc_pv
                        )
                        pool_last = nc.gpsimd.scalar_tensor_tensor(
                            out, xsh(g, dh, dw, r_pool, POOL_ROWS),
                            wap(g, dh, dw), sc_pv, MULT, ADD,
                        )
                else:
                    sc_q = scp.tile([128, POOL_ROWS * 64], F32, tag="scq")
                    sc_qv = sc_q[:].rearrange("p (r w) -> p r w", w=64)
                    for i, (dh, dw) in enumerate(rest):
                        out = (
                            yv(r_pool, POOL_ROWS)
                            if i == len(rest) - 1 else sc_pv
                        )
                        nc.gpsimd.tensor_scalar(
                            sc_qv, xsh(g, dh, dw, r_pool, POOL_ROWS),
                            wap(g, dh, dw), None, MULT,
                        )
                        pool_last = nc.gpsimd.tensor_tensor(
                            out, sc_pv, sc_qv, ADD
                        )
            conv_state[g]["dve_last"] = dve_last
            conv_state[g]["pool_last"] = pool_last

        def pe_conv(g):
            st = in_state[g]
            bias = bst[:, g : g + 1]
            y2 = conv_state[g]["y2"]
            diag = st["diag"]
            abs_xa = pe_absorb(ident[:, 0:1], dep=st["last_copy"])
            abs_pd = pe_absorb(ident[:, 0:1], dep=st["pad_last"])
            abs_dg = pe_absorb(ident[:, 0:1], dep=st["diag_done"])
            pe_copy_last = None
            for r0, nr in _pe_chunks():
                Pq = pcv.tile([128, nr * 64], F32, tag="pcv")
                for i, (dh, dw) in enumerate(ALL_TAPS):
                    if SLAB_DT is F32:
                        lhs = diag[:, 128 * i : 128 * (i + 1)].bitcast(F32R)
                        rhs = xsh(g, dh, dw, r0, nr, dt=False).bitcast(F32R)
                    else:
                        lhs = diag[:, 128 * i : 128 * (i + 1)]
                        rhs = xsh(g, dh, dw, r0, nr, dt=False)
                    mm = nc.tensor.matmul(
                        Pq[:], lhs, rhs,
                        start=(i == 0), stop=(i == len(ALL_TAPS) - 1),
                    )
                    pin_dep(mm, abs_xa, abs_pd, abs_dg)
                pe_copy_last = nc.scalar.activation(
                    y2[:, r0 * 64 : (r0 + nr) * 64].rearrange(
                        "p (r w) -> p r w", w=64
                    ),
                    Pq[:].rearrange("p (r w) -> p r w", w=64),
                    IDENT_F, bias=bias,
                )
            conv_state[g]["pe_copy_last"] = pe_copy_last

        def out_path(g):
            st = conv_state.pop(g)
            y2 = st["y2"]
            yd = ydp.tile([128, 32, 128], F32, tag="yd")
            a1 = pe_absorb(ident[:, 0:1], dep=st["dve_last"])
            a2 = pe_absorb(ident[:, 0:1], dep=st["pool_last"])
            a3 = pe_absorb(ident[:, 0:1], dep=st["pe_copy_last"])
            for q in range(8):
                pt = pout.tile([128, 512], Y2_DT if Y2_DT is BF16 else F32,
                               tag="pout")
                for jo in range(4):
                    j = 4 * q + jo
                    yblk = y2[:, 128 * j : 128 * (j + 1)]
                    if Y2_DT is F32R:
                        t = nc.tensor.transpose(
                            pt[:, 128 * jo : 128 * (jo + 1)].bitcast(F32R),
                            yblk, identr[:],
                        )
                    elif Y2_DT is BF16:
                        t = nc.tensor.transpose(
                            pt[:, 128 * jo : 128 * (jo + 1)], yblk, identb16[:]
                        )
                    else:
                        t = nc.tensor.transpose(
                            pt[:, 128 * jo : 128 * (jo + 1)], yblk, ident[:]
                        )
                    pin_dep(t, a1, a2, a3)
                nc.scalar.copy(
                    yd[:, 4 * q : 4 * q + 4, :],
                    pt[:].rearrange("p (j c) -> p j c", j=4),
                )
            dst = ys[:, :, 2 * g : 2 * g + 2, :].rearrange(
                "(j ph) w dp c -> (ph w) j (dp c)", ph=2
            )
            nc.sync.dma_start(dst, yd[:])

        for p in range(G + CONV_LAG + 1):
            if p < G:
                dma_issue(p)
            if TRANS_LAG <= p < G + TRANS_LAG:
                trans_copy(p - TRANS_LAG)
            if CONV_LAG <= p < G + CONV_LAG:
                g = p - CONV_LAG
                chains(g)
                if g >= OUT_OFF:
                    out_path(g - OUT_OFF)
                pe_conv(g)
        for g in range(G - OUT_OFF, G):
            out_path(g)

    return nc


# walrus setupSyncWait caps per engine struct: PE Matmult takes 1 sem wait,
# ACT/DVE/Pool compute ops take 2. Tile sometimes attaches more (psum slot
# release-sets). Hoist the excess onto injected same-engine Drains (Tile's
# own epilogue Drain carries 12 waits, so Drain accepts many).
_WAIT_CAPS = {"PE": 1, "Activation": 1, "DVE": 1, "Pool": 1, "SP": 1}
_SPLIT_SEQ = [0]


def _split_waits(nc):
    fn = nc.m.functions[0]
    nsplit = 0
    for blk in fn.blocks:
        out = []
        changed = False
        for ins in blk.instructions:
            si = ins.sync_info
            waits = list(si.on_wait) if si is not None and si.on_wait else []
            eng = getattr(ins, "engine", None)
            engname = getattr(eng, "value", None) or str(eng)
            cap = _WAIT_CAPS.get(engname)
            if cap is not None and len(waits) > cap:
                excess, keep = waits[:-cap], waits[-cap:]
                for w in excess:
                    _SPLIT_SEQ[0] += 1
                    d = mybir.InstDrain(name=f"I-ws{_SPLIT_SEQ[0]}", ins=[], outs=[])
                    d.engine = eng
                    d.sync_info = mybir.SyncInfo(on_wait=[w], on_update=[])
                    out.append(d)
                ins.sync_info = mybir.SyncInfo(
                    on_wait=keep, on_update=list(si.on_update or [])
                )
                changed = True
                nsplit += 1
            out.append(ins)
        if changed:
            blk.instructions = out
    return nsplit


_NC_CACHE = None


def _get_nc():
    global _NC_CACHE
    if _NC_CACHE is None:
        nc = _build_nc()
        _split_waits(nc)
        _NC_CACHE = nc
    return _NC_CACHE


class Runner:
    """Persistent PJRT executor for an SPMD bass module (axon path).

    Mirrors bass2jax.run_bass_via_pjrt's multi-core branch but keeps the
    jitted callable so repeated (timed) invocations don't recompile.
    """

    def __init__(self, nc, n_cores=8):
        import jax
        from jax.experimental.shard_map import shard_map
        from jax.sharding import Mesh, PartitionSpec
        from concourse import bass2jax

        bass2jax.install_neuronx_cc_hook()
        self.jax = jax
        self.nc = nc
        self.n = n_cores
        partition_name = (
            nc.partition_id_tensor.name if nc.partition_id_tensor else None
        )
        in_names, out_names, out_avals = [], [], []
        for alloc in nc.m.functions[0].allocations:
            if not isinstance(alloc, mybir.MemoryLocationSet):
                continue
            name = alloc.memorylocations[0].name
            if alloc.kind == "ExternalInput":
                if name != partition_name:
                    in_names.append(name)
            elif alloc.kind == "ExternalOutput":
                out_names.append(name)
                out_avals.append(
                    jax.core.ShapedArray(
                        tuple(alloc.tensor_shape), mybir.dt.np(alloc.dtype)
                    )
                )
        self.in_names = list(in_names)
        self.out_names = out_names
        self.out_avals = out_avals
        bind_in_names = list(in_names) + list(out_names)
        if partition_name is not None:
            bind_in_names.append(partition_name)
        bind_in_names = tuple(bind_in_names)
        n_params = len(in_names)
        n_outs = len(out_names)

        def _body(*args):
            operands = list(args)
            if partition_name is not None:
                operands.append(bass2jax.partition_id_tensor())
            outs = bass2jax._bass_exec_p.bind(
                *operands,
                out_avals=tuple(out_avals),
                in_names=bind_in_names,
                out_names=tuple(out_names),
                lowering_input_output_aliases=(),
                sim_require_finite=True,
                sim_require_nnan=True,
                nc=nc,
            )
            return tuple(outs)

        devices = jax.devices()[:n_cores]
        self.mesh = Mesh(np.asarray(devices), ("core",))
        self.spec = PartitionSpec("core")
        in_specs = (self.spec,) * (n_params + n_outs)
        out_specs = (self.spec,) * n_outs
        donate = tuple(range(n_params, n_params + n_outs))
        self.fn = jax.jit(
            shard_map(
                _body,
                mesh=self.mesh,
                in_specs=in_specs,
                out_specs=out_specs,
                check_rep=False,
            ),
            donate_argnums=donate,
            keep_unused=True,
        )
        sharding = jax.sharding.NamedSharding(self.mesh, self.spec)
        self.zeros_fn = jax.jit(
            lambda: tuple(
                self.jax.numpy.zeros((n_cores * a.shape[0], *a.shape[1:]), a.dtype)
                for a in out_avals
            ),
            out_shardings=(sharding,) * n_outs,
        )

    def put_inputs(self, in_maps):
        """in_maps: per-core dict name->np.ndarray. Returns device arrays."""
        jax = self.jax
        sharding = jax.sharding.NamedSharding(self.mesh, self.spec)
        arrs = []
        for name in self.in_names:
            cat = np.concatenate([np.asarray(m[name]) for m in in_maps], axis=0)
            arrs.append(jax.device_put(cat, sharding))
        jax.block_until_ready(arrs)
        return arrs

    def __call__(self, dev_inputs):
        zs = self.zeros_fn()
        self.jax.block_until_ready(zs)
        out = self.fn(*dev_inputs, *zs)
        self.jax.block_until_ready(out)
        return out

    def time_it(self, dev_inputs, reps=10):
        import time as _t

        ts = []
        for _ in range(reps):
            zs = self.zeros_fn()
            self.jax.block_until_ready(zs)
            t0 = _t.perf_counter()
            out = self.fn(*dev_inputs, *zs)
            self.jax.block_until_ready(out)
            ts.append(_t.perf_counter() - t0)
        return ts

    def to_numpy(self, out):
        n = self.n
        return [
            {
                name: np.asarray(out[i]).reshape(n, *self.out_avals[i].shape)[c]
                for i, name in enumerate(self.out_names)
            }
            for c in range(n)
        ]


_RUNNER = None


def _get_runner():
    global _RUNNER
    if _RUNNER is None:
        _RUNNER = Runner(_get_nc(), B)
    return _RUNNER


def _prep_wb(w, b):
    # ws[p, g*9 + kh*3 + kw] = w[2g + p//64, kh, kw, p%64]
    w = np.asarray(w, dtype=np.float32).reshape(G, 2, 9, C)  # (g, dp, tap, c)
    ws = np.ascontiguousarray(w.transpose(1, 3, 0, 2).reshape(128, G * 9))
    b = np.asarray(b, dtype=np.float32).reshape(G, 2, C)
    bs = np.ascontiguousarray(b.transpose(1, 2, 0).reshape(128, G))
    return ws, bs


def _in_maps(inputs):
    x = np.asarray(inputs["x"], dtype=np.float32)
    ws, bs = _prep_wb(inputs["w"], inputs["b"])
    return [{"xs": np.ascontiguousarray(x[i]), "ws": ws, "bs": bs} for i in range(B)]


def kernel(**inputs) -> np.ndarray:
    r = _get_runner()
    dev = r.put_inputs(_in_maps(inputs))
    res = r.to_numpy(r(dev))
    return np.stack([m["ys"] for m in res], axis=0)


# revision 44
# speedup vs baseline: 2.6436x; 1.1122x over previous
"""Depthwise 3x3 conv over each depth slice of x[B,H,W,D,C] on 8 trn2 cores.

Strategy (v3 — engine-balanced range split, software-pipelined):
  - Data-parallel over batch: core i handles x[i] ([H,W,D,C] = [64,64,32,64]).
  - Per core, loop over 16 depth-pair groups; partitions = (d_parity, C) = 128,
    free axis = spatial (H*W) so per-(d,c) tap weights are per-partition
    scalars.
  - HBM has C contiguous, so the (spatial, channel) <-> (channel, spatial)
    layout change is done on-chip with PE transposes (128x128 blocks, f32r
    in / bf16 out at 1.5 / 1 cycles per row).
  - SAME zero padding handled by a 65-stride padded slab with zeroed guard
    rows/pad column so every tap is a flat shifted read.
  - The 64 spatial rows of each group are split across three engines so all
    of PE / DVE / ACT / Pool run ~13.5us per group:
      rows [0, PE_ROWS):            9 diag-matmuls (f32r, 1 cyc/row)
                                    accumulating in PSUM; ACT copies out,
                                    folding the bias via Identity-activation.
      rows [PE_ROWS, +DVE_ROWS):    tensor_scalar head (w*x+b, 2x mode)
                                    then 8 fused scalar_tensor_tensor taps,
                                    all on DVE.
      rows [.., 64):                head + 8 taps as TS-mult + TT-add pairs
                                    on GPSIMD (walrus rejects STT on Pool).
  - Diag matrices are built per group in ONE broadcast tensor_tensor on
    Pool; the conv result y2 is bf16 (halves out-transpose cost; ~2e-3
    rounding vs the 2e-2 gate; the psum->yd copy upcasts back to f32).
  - Phase pipeline per iteration p: DMA(p) + transposes/copies(p) are
    emitted one iteration ahead of the tap chains / PE conv (p-1), whose
    out-path trails one more group (p-2): PE's in-order stream
    [in-tr(p) | out-tr(p-2) | conv-mm(p-1)] then runs with zero stalls.
  - Sem-wait caps (PE Matmult takes 1 wait) are handled entirely by the
    _split_waits post-pass, which hoists excess waits onto same-engine
    Drain instructions.
"""

import os
from contextlib import ExitStack

import numpy as np

import concourse.bass as bass
import concourse.mybir as mybir
import concourse.tile as tile
from concourse.bass_utils import run_bass_kernel_spmd
from concourse.masks import make_identity
from concourse.tile import add_dep_helper

F32 = mybir.dt.float32
F32R = mybir.dt.float32r
BF16 = mybir.dt.bfloat16

B, H, W, D, C = 8, 64, 64, 32, 64
G = D // 2              # 16 depth-pair groups per core
RS = W + 1              # 65: padded row stride (col 64 of each row is zero)
DATA0 = RS + 1          # 66: flat offset of (h=0, w=0) in the slab
SLAB = DATA0 + 64 * RS + RS + 1   # 66 + 4160 + 66 = 4292
CONVL = 64 * RS         # 4160 = span of a [64 rows x 65] view

MULT = mybir.AluOpType.mult
ADD = mybir.AluOpType.add
IDENT_F = mybir.ActivationFunctionType.Identity

ALL_TAPS = [(dh, dw) for dh in (-1, 0, 1) for dw in (-1, 0, 1)]

# ---- tunables -----------------------------------------------------------
PE_ROWS = 37            # rows on the TensorEngine (diag-matmul taps)
DVE_ROWS = 22           # rows on the vector engine (STT chain)
POOL_ROWS = 64 - PE_ROWS - DVE_ROWS
POOL_STT = False        # walrus rejects STT on Pool -> TS-mult + TT-add
SLAB_DT = F32R          # slab/diag dtype fed to PE matmuls (1 cyc/row)
TRANS_F32R = True       # run the 128x128 transposes as f32r (1.5 vs 2 cyc)
Y2_DT = BF16            # conv result dtype (bf16 = 1cyc/row out-transposes)
ABSORBERS = True        # PE observer matmuls (cheap; shields 1-wait cap)
DIAG_ON_POOL = True     # diag built on gpsimd (DVE fallback)
TRANS_LAG = 0           # iterations between dma issue and transposes
CONV_LAG = 1            # iterations between dma issue and conv
OUT_OFF = 1             # groups between conv and its out-path
PIN_W = 4               # transposes per in-psum tile (4 or 8)
XD_BUFS = 3
XA_BUFS = 3
SC_BUFS = 2
Y2_BUFS = 2
YD_BUFS = 2
PIN_BUFS = 3
PCV_BUFS = 2
POUT_BUFS = 2
POUT_W = 1              # 128-blocks per out-psum tile / 4
LAST_ROWS = (37, 22, 5)  # row split for the final group (drain latency)
FIRST_ROWS = (37, 22, 5)  # row split for group 0 (ramp latency)
LAST2_ROWS = (37, 22, 5)  # row split for group G-2
OUT_POS = 'mid'          # out-path emission: early/mid/late in the iteration
DVE_HEAD_ACT = False     # compute the DVE-range head on ACT instead


def _rows_for(g):
    """(pe, dve, pool) rows for group g; the last group biases toward PE
    to shorten its serial DVE chain (it is the pipeline drain)."""
    if g == G - 1:
        return LAST_ROWS
    if g == G - 2:
        return LAST2_ROWS
    if g == 0:
        return FIRST_ROWS
    return (PE_ROWS, DVE_ROWS, POOL_ROWS)


def _pe_chunks(pe_rows):
    """Split pe_rows into psum-sized chunks (>=4 rows keeps f32r fast)."""
    out, r = [], 0
    while r < pe_rows:
        n = min(8, pe_rows - r)
        out.append((r, n))
        r += n
    assert all(n >= 4 for _, n in out)
    return out


def _build_nc():
    nc = bass.Bass("TRN2", target_bir_lowering=False, debug=False)
    xs = nc.dram_tensor("xs", [H, W, D, C], F32, kind="ExternalInput").ap()
    ws = nc.dram_tensor("ws", [128, G * 9], F32, kind="ExternalInput").ap()
    bs = nc.dram_tensor("bs", [128, G], F32, kind="ExternalInput").ap()
    ys = nc.dram_tensor("ys", [H, W, D, C], F32, kind="ExternalOutput").ap()

    with tile.TileContext(nc) as tc, ExitStack() as ctx:
        consts = ctx.enter_context(tc.tile_pool(name="consts", bufs=1))
        ident = consts.tile([128, 128], F32)
        make_identity(nc, ident[:])
        # f32r identity for the transposes: walrus wants every compute
        # producer feeding an f32r matmul to declare an f32r-rounded output,
        # so materialize it via an ACT copy (f32r out is legal on ACT).
        identr = consts.tile([128, 128], F32R)
        nc.scalar.copy(identr[:], ident[:])
        identb16 = consts.tile([128, 128], BF16)
        nc.scalar.copy(identb16[:], ident[:])
        wst = consts.tile([128, G * 9], F32)
        nc.sync.dma_start(wst[:], ws)
        bst = consts.tile([128, G], F32)
        nc.sync.dma_start(bst[:], bs)

        xdp = ctx.enter_context(tc.tile_pool(name="xd", bufs=XD_BUFS))
        xap = ctx.enter_context(tc.tile_pool(name="xa", bufs=XA_BUFS))
        dgp = ctx.enter_context(tc.tile_pool(name="diag", bufs=3))
        scp = ctx.enter_context(tc.tile_pool(name="scr", bufs=SC_BUFS))
        y2p = ctx.enter_context(tc.tile_pool(name="y2", bufs=Y2_BUFS))
        ydp = ctx.enter_context(tc.tile_pool(name="yd", bufs=YD_BUFS))
        pin = ctx.enter_context(
            tc.tile_pool(name="pin", bufs=PIN_BUFS, space=bass.MemorySpace.PSUM)
        )
        pcv = ctx.enter_context(
            tc.tile_pool(name="pcv", bufs=PCV_BUFS, space=bass.MemorySpace.PSUM)
        )
        pout = ctx.enter_context(
            tc.tile_pool(name="pout", bufs=POUT_BUFS, space=bass.MemorySpace.PSUM)
        )
        dummy = None
        if ABSORBERS:
            pdum = ctx.enter_context(
                tc.tile_pool(name="pdum", bufs=1, space=bass.MemorySpace.PSUM)
            )
            dummy = pdum.tile([128, 8], F32)

        def pe_absorb(col, dep=None):
            if not ABSORBERS:
                return None
            if col.dtype is not F32:
                col = col.bitcast(F32)
            mm = nc.tensor.matmul(
                dummy[0:1, 0:1], col, ident[:, 0:1], skip_group_check=True
            )
            if dep is not None:
                add_dep_helper(mm.ins, dep.ins, reason="observe")
            return mm

        def pin_dep(mm, *deps):
            for d in deps:
                if d is not None:
                    add_dep_helper(mm.ins, d.ins, reason="after-observer")

        pe_absorb(ident[:, 0:1])  # PE observes the identity build once

        # ---- software pipeline: in-path runs TWO groups ahead of the
        # conv so the PE matmuls never wait on the slab handoff (ACT
        # copies) inside PE's in-order stream, and the tap chains start
        # each cycle with their slab already resident.
        in_state = {}
        conv_state = {}

        def dma_issue(g):
            src = xs[:, :, 2 * g : 2 * g + 2, :].rearrange(
                "(j ph) w dp c -> (ph w) j (dp c)", ph=2
            )
            # xd is f32r-typed so the DMA itself is the declared f32r
            # producer for the f32r in-transposes (same bits as f32)
            xd = xdp.tile([128, 32, 128], F32R if TRANS_F32R else F32, tag="xd")
            sv = src if not TRANS_F32R else src.bitcast(F32R)
            if g == 0:
                nc.sync.dma_start(xd[:, 0:8, :], sv[:, 0:8, :])
                nc.sync.dma_start(xd[:, 8:32, :], sv[:, 8:32, :])
            else:
                nc.sync.dma_start(xd[:], sv)
            in_state[g] = dict(xd=xd)

        def trans_copy(g):
            xd = in_state[g]["xd"]
            xa = xap.tile([128, SLAB], SLAB_DT, tag="xa")
            xaf = xa[:] if SLAB_DT is F32 else xa[:].bitcast(F32)
            nc.gpsimd.memset(xaf[:, 0:DATA0], 0.0)
            nc.gpsimd.memset(xaf[:, DATA0 + 63 * RS + 64 : SLAB], 0.0)
            padcol = xaf[:, DATA0 + 64 : DATA0 + 64 + CONVL].rearrange(
                "p (r o) -> p r o", o=RS
            )[:, :, 0:1]
            pad_last = nc.gpsimd.memset(padcol, 0.0)

            absA = pe_absorb(xd[:, 0, 0:1])  # PE observes xd's DMA
            last_copy = None
            # tail (DVE/Pool range) chunks first so their chains can start
            nin = 32 // PIN_W
            order = list(range(nin // 2, nin)) + list(range(nin // 2))
            for k in order:
                pt = pin.tile([128, 128 * PIN_W], F32, tag="pin")
                for jo in range(PIN_W):
                    j = PIN_W * k + jo
                    if TRANS_F32R:
                        t = nc.tensor.transpose(
                            pt[:, 128 * jo : 128 * (jo + 1)].bitcast(F32R),
                            xd[:, j, :],
                            identr[:],
                        )
                    else:
                        t = nc.tensor.transpose(
                            pt[:, 128 * jo : 128 * (jo + 1)], xd[:, j, :], ident[:]
                        )
                    pin_dep(t, absA)
                span = 130 * PIN_W
                dst = xa[:, DATA0 + span * k : DATA0 + span * k + span].rearrange(
                    "p (j r b) -> p j r b", j=PIN_W, b=RS
                )[:, :, :, 0:64]
                srcp = pt[:].rearrange("p (j r b) -> p j r b", j=PIN_W, b=64)
                last_copy = nc.scalar.copy(dst, srcp)

            # diag for the PE taps: ONE broadcast tensor_tensor (on Pool,
            # which has slack; f32r out keeps walrus's producer rule happy)
            diag = dgp.tile([128, 9 * 128], SLAB_DT, tag="diag")
            identb = ident[:].unsqueeze(1).broadcast_to([128, 9, 128])
            wb = (
                wst[:, g * 9 : g * 9 + 9]
                .unsqueeze(2)
                .broadcast_to([128, 9, 128])
            )
            dgv = diag[:].rearrange("p (t c) -> p t c", t=9)
            if DIAG_ON_POOL:
                diag_done = nc.gpsimd.tensor_tensor(dgv, identb, wb, MULT)
            else:
                diag_done = nc.vector.tensor_tensor(dgv, identb, wb, MULT)

            in_state[g].update(
                xa=xa, diag=diag, last_copy=last_copy, pad_last=pad_last,
                diag_done=diag_done,
            )

        def xsh(g, dh, dw, r0, nr, dt=True):
            xa = in_state[g]["xa"]
            s0 = DATA0 + dh * RS + dw + r0 * RS
            v = xa[:, s0 : s0 + nr * RS]
            if dt and SLAB_DT is not F32:
                v = v.bitcast(F32)
            return v.rearrange("p (r b) -> p r b", b=RS)[:, :, 0:64]

        def wap(g, dh, dw):
            i = g * 9 + (dh + 1) * 3 + (dw + 1)
            return wst[:, i : i + 1]

        def chains(g):
            bias = bst[:, g : g + 1]
            y2 = y2p.tile([128, 4096], Y2_DT, tag="y2")
            conv_state[g] = dict(y2=y2)

            def yv(r0, nr):
                return y2[:, r0 * 64 : (r0 + nr) * 64].rearrange(
                    "p (r w) -> p r w", w=64
                )

            pe_r, dve_r, pool_r = _rows_for(g)
            r_dve = pe_r
            r_pool = pe_r + dve_r
            (h0, w0), rest = ALL_TAPS[0], ALL_TAPS[1:]
            sc_d = scp.tile([128, DVE_ROWS * 64], F32, tag="scd")
            sc_dv = sc_d[:, 0 : dve_r * 64].rearrange("p (r w) -> p r w", w=64)
            if DVE_HEAD_ACT:
                nc.scalar.activation(
                    sc_dv, xsh(g, h0, w0, r_dve, dve_r), IDENT_F,
                    bias=bias, scale=wap(g, h0, w0),
                )
            else:
                nc.vector.tensor_scalar(
                    sc_dv, xsh(g, h0, w0, r_dve, dve_r), wap(g, h0, w0), bias,
                    MULT, ADD,
                )
            dve_last = None
            for i, (dh, dw) in enumerate(rest):
                out = yv(r_dve, dve_r) if i == len(rest) - 1 else sc_dv
                dve_last = nc.vector.scalar_tensor_tensor(
                    out, xsh(g, dh, dw, r_dve, dve_r), wap(g, dh, dw),
                    sc_dv, MULT, ADD,
                )

            pool_last = None
            if pool_r:
                sc_p = scp.tile([128, max(POOL_ROWS, LAST_ROWS[2]) * 64], F32,
                                tag="scp")
                sc_pv = sc_p[:, 0 : pool_r * 64].rearrange(
                    "p (r w) -> p r w", w=64
                )
                nc.gpsimd.tensor_scalar(
                    sc_pv, xsh(g, h0, w0, r_pool, pool_r), wap(g, h0, w0),
                    bias, MULT, ADD,
                )
                sc_q = scp.tile([128, max(POOL_ROWS, LAST_ROWS[2]) * 64], F32,
                                tag="scq")
                sc_qv = sc_q[:, 0 : pool_r * 64].rearrange(
                    "p (r w) -> p r w", w=64
                )
                for i, (dh, dw) in enumerate(rest):
                    out = yv(r_pool, pool_r) if i == len(rest) - 1 else sc_pv
                    nc.gpsimd.tensor_scalar(
                        sc_qv, xsh(g, dh, dw, r_pool, pool_r),
                        wap(g, dh, dw), None, MULT,
                    )
                    pool_last = nc.gpsimd.tensor_tensor(out, sc_pv, sc_qv, ADD)
            conv_state[g]["dve_last"] = dve_last
            conv_state[g]["pool_last"] = pool_last

        def pe_conv(g):
            st = in_state[g]
            bias = bst[:, g : g + 1]
            y2 = conv_state[g]["y2"]
            diag = st["diag"]
            abs_xa = pe_absorb(ident[:, 0:1], dep=st["last_copy"])
            abs_pd = pe_absorb(ident[:, 0:1], dep=st["pad_last"])
            abs_dg = pe_absorb(ident[:, 0:1], dep=st["diag_done"])
            pe_copy_last = None
            for r0, nr in _pe_chunks(_rows_for(g)[0]):
                Pq = pcv.tile([128, nr * 64], F32, tag="pcv")
                for i, (dh, dw) in enumerate(ALL_TAPS):
                    if SLAB_DT is F32:
                        lhs = diag[:, 128 * i : 128 * (i + 1)].bitcast(F32R)
                        rhs = xsh(g, dh, dw, r0, nr, dt=False).bitcast(F32R)
                    else:
                        lhs = diag[:, 128 * i : 128 * (i + 1)]
                        rhs = xsh(g, dh, dw, r0, nr, dt=False)
                    mm = nc.tensor.matmul(
                        Pq[:], lhs, rhs,
                        start=(i == 0), stop=(i == len(ALL_TAPS) - 1),
                    )
                    pin_dep(mm, abs_xa, abs_pd, abs_dg)
                pe_copy_last = nc.scalar.activation(
                    y2[:, r0 * 64 : (r0 + nr) * 64].rearrange(
                        "p (r w) -> p r w", w=64
                    ),
                    Pq[:].rearrange("p (r w) -> p r w", w=64),
                    IDENT_F, bias=bias,
                )
            conv_state[g]["pe_copy_last"] = pe_copy_last

        def out_path(g):
            st = conv_state.pop(g)
            y2 = st["y2"]
            yd = ydp.tile([128, 32, 128], F32, tag="yd")
            a1 = pe_absorb(ident[:, 0:1], dep=st["dve_last"])
            a2 = pe_absorb(ident[:, 0:1], dep=st["pool_last"])
            a3 = pe_absorb(ident[:, 0:1], dep=st["pe_copy_last"])
            for q in range(8 // POUT_W):
                pt = pout.tile([128, 512 * POUT_W],
                               Y2_DT if Y2_DT is BF16 else F32, tag="pout")
                for jo in range(4 * POUT_W):
                    j = 4 * POUT_W * q + jo
                    yblk = y2[:, 128 * j : 128 * (j + 1)]
                    if Y2_DT is F32R:
                        t = nc.tensor.transpose(
                            pt[:, 128 * jo : 128 * (jo + 1)].bitcast(F32R),
                            yblk, identr[:],
                        )
                    elif Y2_DT is BF16:
                        t = nc.tensor.transpose(
                            pt[:, 128 * jo : 128 * (jo + 1)], yblk, identb16[:]
                        )
                    else:
                        t = nc.tensor.transpose(
                            pt[:, 128 * jo : 128 * (jo + 1)], yblk, ident[:]
                        )
                    pin_dep(t, a1, a2, a3)
                nc.scalar.copy(
                    yd[:, 4 * POUT_W * q : 4 * POUT_W * (q + 1), :],
                    pt[:].rearrange("p (j c) -> p j c", j=4 * POUT_W),
                )
            dst = ys[:, :, 2 * g : 2 * g + 2, :].rearrange(
                "(j ph) w dp c -> (ph w) j (dp c)", ph=2
            )
            if g == G - 1:
                nc.sync.dma_start(dst[:, 0:16, :], yd[:, 0:16, :])
                nc.sync.dma_start(dst[:, 16:32, :], yd[:, 16:32, :])
            else:
                nc.sync.dma_start(dst, yd[:])

        for p in range(G + CONV_LAG + 1):
            if p < G:
                dma_issue(p)
            if TRANS_LAG <= p < G + TRANS_LAG:
                trans_copy(p - TRANS_LAG)
            if CONV_LAG <= p < G + CONV_LAG:
                g = p - CONV_LAG
                if OUT_POS == 'early' and g >= OUT_OFF:
                    out_path(g - OUT_OFF)
                chains(g)
                if OUT_POS == 'mid' and g >= OUT_OFF:
                    out_path(g - OUT_OFF)
                pe_conv(g)
                if OUT_POS == 'late' and g >= OUT_OFF:
                    out_path(g - OUT_OFF)
        for g in range(G - OUT_OFF, G):
            out_path(g)

    return nc


# walrus setupSyncWait caps per engine struct: PE Matmult takes 1 sem wait,
# ACT/DVE/Pool compute ops take 2. Tile sometimes attaches more (psum slot
# release-sets). Hoist the excess onto injected same-engine Drains (Tile's
# own epilogue Drain carries 12 waits, so Drain accepts many).
_WAIT_CAPS = {"PE": 1, "Activation": 1, "DVE": 1, "Pool": 1, "SP": 1}
_SPLIT_SEQ = [0]


def _split_waits(nc):
    fn = nc.m.functions[0]
    nsplit = 0
    for blk in fn.blocks:
        out = []
        changed = False
        for ins in blk.instructions:
            si = ins.sync_info
            waits = list(si.on_wait) if si is not None and si.on_wait else []
            eng = getattr(ins, "engine", None)
            engname = getattr(eng, "value", None) or str(eng)
            cap = _WAIT_CAPS.get(engname)
            if cap is not None and len(waits) > cap:
                excess, keep = waits[:-cap], waits[-cap:]
                for w in excess:
                    _SPLIT_SEQ[0] += 1
                    d = mybir.InstDrain(name=f"I-ws{_SPLIT_SEQ[0]}", ins=[], outs=[])
                    d.engine = eng
                    d.sync_info = mybir.SyncInfo(on_wait=[w], on_update=[])
                    out.append(d)
                ins.sync_info = mybir.SyncInfo(
                    on_wait=keep, on_update=list(si.on_update or [])
                )
                changed = True
                nsplit += 1
            out.append(ins)
        if changed:
            blk.instructions = out
    return nsplit


_NC_CACHE = None


def _get_nc():
    global _NC_CACHE
    if _NC_CACHE is None:
        nc = _build_nc()
        _split_waits(nc)
        _NC_CACHE = nc
    return _NC_CACHE


class Runner:
    """Persistent PJRT executor for an SPMD bass module (axon path).

    Mirrors bass2jax.run_bass_via_pjrt's multi-core branch but keeps the
    jitted callable so repeated (timed) invocations don't recompile.
    """

    def __init__(self, nc, n_cores=8):
        import jax
        from jax.experimental.shard_map import shard_map
        from jax.sharding import Mesh, PartitionSpec
        from concourse import bass2jax

        bass2jax.install_neuronx_cc_hook()
        self.jax = jax
        self.nc = nc
        self.n = n_cores
        partition_name = (
            nc.partition_id_tensor.name if nc.partition_id_tensor else None
        )
        in_names, out_names, out_avals = [], [], []
        for alloc in nc.m.functions[0].allocations:
            if not isinstance(alloc, mybir.MemoryLocationSet):
                continue
            name = alloc.memorylocations[0].name
            if alloc.kind == "ExternalInput":
                if name != partition_name:
                    in_names.append(name)
            elif alloc.kind == "ExternalOutput":
                out_names.append(name)
                out_avals.append(
                    jax.core.ShapedArray(
                        tuple(alloc.tensor_shape), mybir.dt.np(alloc.dtype)
                    )
                )
        self.in_names = list(in_names)
        self.out_names = out_names
        self.out_avals = out_avals
        bind_in_names = list(in_names) + list(out_names)
        if partition_name is not None:
            bind_in_names.append(partition_name)
        bind_in_names = tuple(bind_in_names)
        n_params = len(in_names)
        n_outs = len(out_names)

        def _body(*args):
            operands = list(args)
            if partition_name is not None:
                operands.append(bass2jax.partition_id_tensor())
            outs = bass2jax._bass_exec_p.bind(
                *operands,
                out_avals=tuple(out_avals),
                in_names=bind_in_names,
                out_names=tuple(out_names),
                lowering_input_output_aliases=(),
                sim_require_finite=True,
                sim_require_nnan=True,
                nc=nc,
            )
            return tuple(outs)

        devices = jax.devices()[:n_cores]
        self.mesh = Mesh(np.asarray(devices), ("core",))
        self.spec = PartitionSpec("core")
        in_specs = (self.spec,) * (n_params + n_outs)
        out_specs = (self.spec,) * n_outs
        donate = tuple(range(n_params, n_params + n_outs))
        self.fn = jax.jit(
            shard_map(
                _body,
                mesh=self.mesh,
                in_specs=in_specs,
                out_specs=out_specs,
                check_rep=False,
            ),
            donate_argnums=donate,
            keep_unused=True,
        )
        sharding = jax.sharding.NamedSharding(self.mesh, self.spec)
        self.zeros_fn = jax.jit(
            lambda: tuple(
                self.jax.numpy.zeros((n_cores * a.shape[0], *a.shape[1:]), a.dtype)
                for a in out_avals
            ),
            out_shardings=(sharding,) * n_outs,
        )

    def put_inputs(self, in_maps):
        """in_maps: per-core dict name->np.ndarray. Returns device arrays."""
        jax = self.jax
        sharding = jax.sharding.NamedSharding(self.mesh, self.spec)
        arrs = []
        for name in self.in_names:
            cat = np.concatenate([np.asarray(m[name]) for m in in_maps], axis=0)
            arrs.append(jax.device_put(cat, sharding))
        jax.block_until_ready(arrs)
        return arrs

    def __call__(self, dev_inputs):
        zs = self.zeros_fn()
        self.jax.block_until_ready(zs)
        out = self.fn(*dev_inputs, *zs)
        self.jax.block_until_ready(out)
        return out

    def time_it(self, dev_inputs, reps=10):
        import time as _t

        ts = []
        for _ in range(reps):
            zs = self.zeros_fn()
            self.jax.block_until_ready(zs)
            t0 = _t.perf_counter()
            out = self.fn(*dev_inputs, *zs)
            self.jax.block_until_ready(out)
            ts.append(_t.perf_counter() - t0)
        return ts

    def to_numpy(self, out):
        n = self.n
        return [
            {
                name: np.asarray(out[i]).reshape(n, *self.out_avals[i].shape)[c]
                for i, name in enumerate(self.out_names)
            }
            for c in range(n)
        ]


_RUNNER = None


def _get_runner():
    global _RUNNER
    if _RUNNER is None:
        _RUNNER = Runner(_get_nc(), B)
    return _RUNNER


def _prep_wb(w, b):
    # ws[p, g*9 + kh*3 + kw] = w[2g + p//64, kh, kw, p%64]
    w = np.asarray(w, dtype=np.float32).reshape(G, 2, 9, C)  # (g, dp, tap, c)
    ws = np.ascontiguousarray(w.transpose(1, 3, 0, 2).reshape(128, G * 9))
    b = np.asarray(b, dtype=np.float32).reshape(G, 2, C)
    bs = np.ascontiguousarray(b.transpose(1, 2, 0).reshape(128, G))
    return ws, bs


def _in_maps(inputs):
    x = np.asarray(inputs["x"], dtype=np.float32)
    ws, bs = _prep_wb(inputs["w"], inputs["b"])
    return [{"xs": np.ascontiguousarray(x[i]), "ws": ws, "bs": bs} for i in range(B)]


def kernel(**inputs) -> np.ndarray:
    r = _get_runner()
    dev = r.put_inputs(_in_maps(inputs))
    res = r.to_numpy(r(dev))
    return np.stack([m["ys"] for m in res], axis=0)


# revision 45
# speedup vs baseline: 2.6793x; 1.0135x over previous
"""Depthwise 3x3 conv over each depth slice of x[B,H,W,D,C] on 8 trn2 cores.

Strategy (v3 — engine-balanced range split, software-pipelined):
  - Data-parallel over batch: core i handles x[i] ([H,W,D,C] = [64,64,32,64]).
  - Per core, loop over 16 depth-pair groups; partitions = (d_parity, C) = 128,
    free axis = spatial (H*W) so per-(d,c) tap weights are per-partition
    scalars.
  - HBM has C contiguous, so the (spatial, channel) <-> (channel, spatial)
    layout change is done on-chip with PE transposes (128x128 blocks, f32r
    in / bf16 out at 1.5 / 1 cycles per row).
  - SAME zero padding handled by a 65-stride padded slab with zeroed guard
    rows/pad column so every tap is a flat shifted read.
  - The 64 spatial rows of each group are split across three engines so all
    of PE / DVE / ACT / Pool run ~13.5us per group:
      rows [0, PE_ROWS):            9 diag-matmuls (f32r, 1 cyc/row)
                                    accumulating in PSUM; ACT copies out,
                                    folding the bias via Identity-activation.
      rows [PE_ROWS, +DVE_ROWS):    tensor_scalar head (w*x+b, 2x mode)
                                    then 8 fused scalar_tensor_tensor taps,
                                    all on DVE.
      rows [.., 64):                head + 8 taps as TS-mult + TT-add pairs
                                    on GPSIMD (walrus rejects STT on Pool).
  - Diag matrices are built per group in ONE broadcast tensor_tensor on
    Pool; the conv result y2 is bf16 (halves out-transpose cost; ~2e-3
    rounding vs the 2e-2 gate; the psum->yd copy upcasts back to f32).
  - Phase pipeline per iteration p: DMA(p) + transposes/copies(p) are
    emitted one iteration ahead of the tap chains / PE conv (p-1), whose
    out-path trails one more group (p-2): PE's in-order stream
    [in-tr(p) | out-tr(p-2) | conv-mm(p-1)] then runs with zero stalls.
  - Sem-wait caps (PE Matmult takes 1 wait) are handled entirely by the
    _split_waits post-pass, which hoists excess waits onto same-engine
    Drain instructions.
"""

import os
from contextlib import ExitStack

import numpy as np

import concourse.bass as bass
import concourse.mybir as mybir
import concourse.tile as tile
from concourse.bass_utils import run_bass_kernel_spmd
from concourse.masks import make_identity
from concourse.tile import add_dep_helper

F32 = mybir.dt.float32
F32R = mybir.dt.float32r
BF16 = mybir.dt.bfloat16

B, H, W, D, C = 8, 64, 64, 32, 64
G = D // 2              # 16 depth-pair groups per core
RS = W + 1              # 65: padded row stride (col 64 of each row is zero)
DATA0 = RS + 1          # 66: flat offset of (h=0, w=0) in the slab
SLAB = DATA0 + 64 * RS + RS + 1   # 66 + 4160 + 66 = 4292
CONVL = 64 * RS         # 4160 = span of a [64 rows x 65] view

MULT = mybir.AluOpType.mult
ADD = mybir.AluOpType.add
IDENT_F = mybir.ActivationFunctionType.Identity

ALL_TAPS = [(dh, dw) for dh in (-1, 0, 1) for dw in (-1, 0, 1)]

# ---- tunables -----------------------------------------------------------
PE_ROWS = 37            # rows on the TensorEngine (diag-matmul taps)
DVE_ROWS = 22           # rows on the vector engine (STT chain)
POOL_ROWS = 64 - PE_ROWS - DVE_ROWS
POOL_STT = False        # walrus rejects STT on Pool -> TS-mult + TT-add
SLAB_DT = F32R          # slab/diag dtype fed to PE matmuls (1 cyc/row)
TRANS_F32R = True       # run the 128x128 transposes as f32r (1.5 vs 2 cyc)
Y2_DT = BF16            # conv result dtype (bf16 = 1cyc/row out-transposes)
ABSORBERS = True        # PE observer matmuls (cheap; shields 1-wait cap)
DIAG_ON_POOL = True     # diag built on gpsimd (DVE fallback)
TRANS_LAG = 0           # iterations between dma issue and transposes
CONV_LAG = 1            # iterations between dma issue and conv
OUT_OFF = 1             # groups between conv and its out-path
PIN_W = 4               # transposes per in-psum tile (4 or 8)
XD_BUFS = 3
XA_BUFS = 3
SC_BUFS = 2
Y2_BUFS = 2
YD_BUFS = 2
PIN_BUFS = 3
PCV_BUFS = 2
POUT_BUFS = 2
POUT_W = 1              # 128-blocks per out-psum tile / 4
LAST_ROWS = (37, 22, 5)  # row split for the final group (drain latency)
FIRST_ROWS = (37, 22, 5)  # row split for group 0 (ramp latency)
LAST2_ROWS = (37, 22, 5)  # row split for group G-2
OUT_POS = 'mid'          # out-path emission: early/mid/late in the iteration
DVE_HEAD_ACT = False     # compute the DVE-range head on ACT instead


def _rows_for(g):
    """(pe, dve, pool) rows for group g; the last group biases toward PE
    to shorten its serial DVE chain (it is the pipeline drain)."""
    if g == G - 1:
        return LAST_ROWS
    if g == G - 2:
        return LAST2_ROWS
    if g == 0:
        return FIRST_ROWS
    return (PE_ROWS, DVE_ROWS, POOL_ROWS)


def _pe_chunks(pe_rows):
    """Split pe_rows into psum-sized chunks (>=4 rows keeps f32r fast)."""
    out, r = [], 0
    while r < pe_rows:
        n = min(8, pe_rows - r)
        out.append((r, n))
        r += n
    assert all(n >= 4 for _, n in out)
    return out


def _build_nc():
    nc = bass.Bass("TRN2", target_bir_lowering=False, debug=False)
    xs = nc.dram_tensor("xs", [H, W, D, C], F32, kind="ExternalInput").ap()
    wbs = nc.dram_tensor("wbs", [128, G * 9 + G], F32, kind="ExternalInput").ap()
    ys = nc.dram_tensor("ys", [H, W, D, C], F32, kind="ExternalOutput").ap()

    with tile.TileContext(nc) as tc, ExitStack() as ctx:
        consts = ctx.enter_context(tc.tile_pool(name="consts", bufs=1))
        ident = consts.tile([128, 128], F32)
        make_identity(nc, ident[:])
        # f32r identity for the transposes: walrus wants every compute
        # producer feeding an f32r matmul to declare an f32r-rounded output,
        # so materialize it via an ACT copy (f32r out is legal on ACT).
        identr = consts.tile([128, 128], F32R)
        nc.scalar.copy(identr[:], ident[:])
        identb16 = consts.tile([128, 128], BF16)
        nc.scalar.copy(identb16[:], ident[:])
        wbst = consts.tile([128, G * 9 + G], F32)
        nc.sync.dma_start(wbst[:], wbs)
        wst = wbst[:, 0 : G * 9]
        bst = wbst[:, G * 9 : G * 9 + G]

        xdp = ctx.enter_context(tc.tile_pool(name="xd", bufs=XD_BUFS))
        xap = ctx.enter_context(tc.tile_pool(name="xa", bufs=XA_BUFS))
        dgp = ctx.enter_context(tc.tile_pool(name="diag", bufs=3))
        scp = ctx.enter_context(tc.tile_pool(name="scr", bufs=SC_BUFS))
        y2p = ctx.enter_context(tc.tile_pool(name="y2", bufs=Y2_BUFS))
        ydp = ctx.enter_context(tc.tile_pool(name="yd", bufs=YD_BUFS))
        pin = ctx.enter_context(
            tc.tile_pool(name="pin", bufs=PIN_BUFS, space=bass.MemorySpace.PSUM)
        )
        pcv = ctx.enter_context(
            tc.tile_pool(name="pcv", bufs=PCV_BUFS, space=bass.MemorySpace.PSUM)
        )
        pout = ctx.enter_context(
            tc.tile_pool(name="pout", bufs=POUT_BUFS, space=bass.MemorySpace.PSUM)
        )
        dummy = None
        if ABSORBERS:
            pdum = ctx.enter_context(
                tc.tile_pool(name="pdum", bufs=1, space=bass.MemorySpace.PSUM)
            )
            dummy = pdum.tile([128, 8], F32)

        def pe_absorb(col, dep=None):
            if not ABSORBERS:
                return None
            if col.dtype is not F32:
                col = col.bitcast(F32)
            mm = nc.tensor.matmul(
                dummy[0:1, 0:1], col, ident[:, 0:1], skip_group_check=True
            )
            if dep is not None:
                add_dep_helper(mm.ins, dep.ins, reason="observe")
            return mm

        def pin_dep(mm, *deps):
            for d in deps:
                if d is not None:
                    add_dep_helper(mm.ins, d.ins, reason="after-observer")

        pe_absorb(ident[:, 0:1])  # PE observes the identity build once

        # ---- software pipeline: in-path runs TWO groups ahead of the
        # conv so the PE matmuls never wait on the slab handoff (ACT
        # copies) inside PE's in-order stream, and the tap chains start
        # each cycle with their slab already resident.
        in_state = {}
        conv_state = {}

        def dma_issue(g):
            src = xs[:, :, 2 * g : 2 * g + 2, :].rearrange(
                "(j ph) w dp c -> (ph w) j (dp c)", ph=2
            )
            # xd is f32r-typed so the DMA itself is the declared f32r
            # producer for the f32r in-transposes (same bits as f32)
            xd = xdp.tile([128, 32, 128], F32R if TRANS_F32R else F32, tag="xd")
            sv = src if not TRANS_F32R else src.bitcast(F32R)
            if g == 0:
                nc.sync.dma_start(xd[:, 0:8, :], sv[:, 0:8, :])
                nc.sync.dma_start(xd[:, 8:32, :], sv[:, 8:32, :])
            else:
                nc.sync.dma_start(xd[:], sv)
            in_state[g] = dict(xd=xd)

        def trans_copy(g):
            xd = in_state[g]["xd"]
            xa = xap.tile([128, SLAB], SLAB_DT, tag="xa")
            pad_last = None
            if g < XA_BUFS:
                # the guard rows / pad column are disjoint from the data
                # copies (which write cols 0:64 of rows 0:64 only), so they
                # persist across buffer reuse: zero each physical buffer once
                xaf = xa[:] if SLAB_DT is F32 else xa[:].bitcast(F32)
                nc.gpsimd.memset(xaf[:, 0:DATA0], 0.0)
                nc.gpsimd.memset(xaf[:, DATA0 + 63 * RS + 64 : SLAB], 0.0)
                padcol = xaf[:, DATA0 + 64 : DATA0 + 64 + CONVL].rearrange(
                    "p (r o) -> p r o", o=RS
                )[:, :, 0:1]
                pad_last = nc.gpsimd.memset(padcol, 0.0)

            absA = pe_absorb(xd[:, 0, 0:1])  # PE observes xd's DMA
            last_copy = None
            # tail (DVE/Pool range) chunks first so their chains can start
            nin = 32 // PIN_W
            order = list(range(nin // 2, nin)) + list(range(nin // 2))
            for k in order:
                pt = pin.tile([128, 128 * PIN_W], F32, tag="pin")
                for jo in range(PIN_W):
                    j = PIN_W * k + jo
                    if TRANS_F32R:
                        t = nc.tensor.transpose(
                            pt[:, 128 * jo : 128 * (jo + 1)].bitcast(F32R),
                            xd[:, j, :],
                            identr[:],
                        )
                    else:
                        t = nc.tensor.transpose(
                            pt[:, 128 * jo : 128 * (jo + 1)], xd[:, j, :], ident[:]
                        )
                    pin_dep(t, absA)
                span = 130 * PIN_W
                dst = xa[:, DATA0 + span * k : DATA0 + span * k + span].rearrange(
                    "p (j r b) -> p j r b", j=PIN_W, b=RS
                )[:, :, :, 0:64]
                srcp = pt[:].rearrange("p (j r b) -> p j r b", j=PIN_W, b=64)
                last_copy = nc.scalar.copy(dst, srcp)

            # diag for the PE taps: ONE broadcast tensor_tensor (on Pool,
            # which has slack; f32r out keeps walrus's producer rule happy)
            diag = dgp.tile([128, 9 * 128], SLAB_DT, tag="diag")
            identb = ident[:].unsqueeze(1).broadcast_to([128, 9, 128])
            wb = (
                wst[:, g * 9 : g * 9 + 9]
                .unsqueeze(2)
                .broadcast_to([128, 9, 128])
            )
            dgv = diag[:].rearrange("p (t c) -> p t c", t=9)
            if DIAG_ON_POOL:
                diag_done = nc.gpsimd.tensor_tensor(dgv, identb, wb, MULT)
            else:
                diag_done = nc.vector.tensor_tensor(dgv, identb, wb, MULT)

            in_state[g].update(
                xa=xa, diag=diag, last_copy=last_copy, pad_last=pad_last,
                diag_done=diag_done,
            )

        def xsh(g, dh, dw, r0, nr, dt=True):
            xa = in_state[g]["xa"]
            s0 = DATA0 + dh * RS + dw + r0 * RS
            v = xa[:, s0 : s0 + nr * RS]
            if dt and SLAB_DT is not F32:
                v = v.bitcast(F32)
            return v.rearrange("p (r b) -> p r b", b=RS)[:, :, 0:64]

        def wap(g, dh, dw):
            i = g * 9 + (dh + 1) * 3 + (dw + 1)
            return wst[:, i : i + 1]

        def chains(g):
            bias = bst[:, g : g + 1]
            y2 = y2p.tile([128, 4096], Y2_DT, tag="y2")
            conv_state[g] = dict(y2=y2)

            def yv(r0, nr):
                return y2[:, r0 * 64 : (r0 + nr) * 64].rearrange(
                    "p (r w) -> p r w", w=64
                )

            pe_r, dve_r, pool_r = _rows_for(g)
            r_dve = pe_r
            r_pool = pe_r + dve_r
            (h0, w0), rest = ALL_TAPS[0], ALL_TAPS[1:]
            sc_d = scp.tile([128, DVE_ROWS * 64], F32, tag="scd")
            sc_dv = sc_d[:, 0 : dve_r * 64].rearrange("p (r w) -> p r w", w=64)
            if DVE_HEAD_ACT:
                nc.scalar.activation(
                    sc_dv, xsh(g, h0, w0, r_dve, dve_r), IDENT_F,
                    bias=bias, scale=wap(g, h0, w0),
                )
            else:
                nc.vector.tensor_scalar(
                    sc_dv, xsh(g, h0, w0, r_dve, dve_r), wap(g, h0, w0), bias,
                    MULT, ADD,
                )
            dve_last = None
            for i, (dh, dw) in enumerate(rest):
                out = yv(r_dve, dve_r) if i == len(rest) - 1 else sc_dv
                dve_last = nc.vector.scalar_tensor_tensor(
                    out, xsh(g, dh, dw, r_dve, dve_r), wap(g, dh, dw),
                    sc_dv, MULT, ADD,
                )

            pool_last = None
            if pool_r:
                sc_p = scp.tile([128, max(POOL_ROWS, LAST_ROWS[2]) * 64], F32,
                                tag="scp")
                sc_pv = sc_p[:, 0 : pool_r * 64].rearrange(
                    "p (r w) -> p r w", w=64
                )
                nc.gpsimd.tensor_scalar(
                    sc_pv, xsh(g, h0, w0, r_pool, pool_r), wap(g, h0, w0),
                    bias, MULT, ADD,
                )
                sc_q = scp.tile([128, max(POOL_ROWS, LAST_ROWS[2]) * 64], F32,
                                tag="scq")
                sc_qv = sc_q[:, 0 : pool_r * 64].rearrange(
                    "p (r w) -> p r w", w=64
                )
                for i, (dh, dw) in enumerate(rest):
                    out = yv(r_pool, pool_r) if i == len(rest) - 1 else sc_pv
                    nc.gpsimd.tensor_scalar(
                        sc_qv, xsh(g, dh, dw, r_pool, pool_r),
                        wap(g, dh, dw), None, MULT,
                    )
                    pool_last = nc.gpsimd.tensor_tensor(out, sc_pv, sc_qv, ADD)
            conv_state[g]["dve_last"] = dve_last
            conv_state[g]["pool_last"] = pool_last

        def pe_conv(g):
            st = in_state[g]
            bias = bst[:, g : g + 1]
            y2 = conv_state[g]["y2"]
            diag = st["diag"]
            abs_xa = pe_absorb(ident[:, 0:1], dep=st["last_copy"])
            abs_pd = pe_absorb(ident[:, 0:1], dep=st["pad_last"])
            abs_dg = pe_absorb(ident[:, 0:1], dep=st["diag_done"])
            pe_copy_last = None
            for r0, nr in _pe_chunks(_rows_for(g)[0]):
                Pq = pcv.tile([128, nr * 64], F32, tag="pcv")
                for i, (dh, dw) in enumerate(ALL_TAPS):
                    if SLAB_DT is F32:
                        lhs = diag[:, 128 * i : 128 * (i + 1)].bitcast(F32R)
                        rhs = xsh(g, dh, dw, r0, nr, dt=False).bitcast(F32R)
                    else:
                        lhs = diag[:, 128 * i : 128 * (i + 1)]
                        rhs = xsh(g, dh, dw, r0, nr, dt=False)
                    mm = nc.tensor.matmul(
                        Pq[:], lhs, rhs,
                        start=(i == 0), stop=(i == len(ALL_TAPS) - 1),
                    )
                    pin_dep(mm, abs_xa, abs_pd, abs_dg)
                pe_copy_last = nc.scalar.activation(
                    y2[:, r0 * 64 : (r0 + nr) * 64].rearrange(
                        "p (r w) -> p r w", w=64
                    ),
                    Pq[:].rearrange("p (r w) -> p r w", w=64),
                    IDENT_F, bias=bias,
                )
            conv_state[g]["pe_copy_last"] = pe_copy_last

        def out_path(g):
            st = conv_state.pop(g)
            y2 = st["y2"]
            yd = ydp.tile([128, 32, 128], F32, tag="yd")
            a1 = pe_absorb(ident[:, 0:1], dep=st["dve_last"])
            a2 = pe_absorb(ident[:, 0:1], dep=st["pool_last"])
            a3 = pe_absorb(ident[:, 0:1], dep=st["pe_copy_last"])
            for q in range(8 // POUT_W):
                pt = pout.tile([128, 512 * POUT_W],
                               Y2_DT if Y2_DT is BF16 else F32, tag="pout")
                for jo in range(4 * POUT_W):
                    j = 4 * POUT_W * q + jo
                    yblk = y2[:, 128 * j : 128 * (j + 1)]
                    if Y2_DT is F32R:
                        t = nc.tensor.transpose(
                            pt[:, 128 * jo : 128 * (jo + 1)].bitcast(F32R),
                            yblk, identr[:],
                        )
                    elif Y2_DT is BF16:
                        t = nc.tensor.transpose(
                            pt[:, 128 * jo : 128 * (jo + 1)], yblk, identb16[:]
                        )
                    else:
                        t = nc.tensor.transpose(
                            pt[:, 128 * jo : 128 * (jo + 1)], yblk, ident[:]
                        )
                    pin_dep(t, a1, a2, a3)
                nc.scalar.copy(
                    yd[:, 4 * POUT_W * q : 4 * POUT_W * (q + 1), :],
                    pt[:].rearrange("p (j c) -> p j c", j=4 * POUT_W),
                )
            dst = ys[:, :, 2 * g : 2 * g + 2, :].rearrange(
                "(j ph) w dp c -> (ph w) j (dp c)", ph=2
            )
            if g == G - 1:
                nc.sync.dma_start(dst[:, 0:16, :], yd[:, 0:16, :])
                nc.sync.dma_start(dst[:, 16:32, :], yd[:, 16:32, :])
            else:
                nc.sync.dma_start(dst, yd[:])

        for p in range(G + CONV_LAG + 1):
            if p < G:
                dma_issue(p)
            if TRANS_LAG <= p < G + TRANS_LAG:
                trans_copy(p - TRANS_LAG)
            if CONV_LAG <= p < G + CONV_LAG:
                g = p - CONV_LAG
                if OUT_POS == 'early' and g >= OUT_OFF:
                    out_path(g - OUT_OFF)
                chains(g)
                if OUT_POS == 'mid' and g >= OUT_OFF:
                    out_path(g - OUT_OFF)
                pe_conv(g)
                if OUT_POS == 'late' and g >= OUT_OFF:
                    out_path(g - OUT_OFF)
        for g in range(G - OUT_OFF, G):
            out_path(g)

    return nc


# walrus setupSyncWait caps per engine struct: PE Matmult takes 1 sem wait,
# ACT/DVE/Pool compute ops take 2. Tile sometimes attaches more (psum slot
# release-sets). Hoist the excess onto injected same-engine Drains (Tile's
# own epilogue Drain carries 12 waits, so Drain accepts many).
_WAIT_CAPS = {"PE": 1, "Activation": 1, "DVE": 1, "Pool": 1, "SP": 1}
_SPLIT_SEQ = [0]


def _split_waits(nc):
    fn = nc.m.functions[0]
    nsplit = 0
    for blk in fn.blocks:
        out = []
        changed = False
        for ins in blk.instructions:
            si = ins.sync_info
            waits = list(si.on_wait) if si is not None and si.on_wait else []
            eng = getattr(ins, "engine", None)
            engname = getattr(eng, "value", None) or str(eng)
            cap = _WAIT_CAPS.get(engname)
            if cap is not None and len(waits) > cap:
                excess, keep = waits[:-cap], waits[-cap:]
                for w in excess:
                    _SPLIT_SEQ[0] += 1
                    d = mybir.InstDrain(name=f"I-ws{_SPLIT_SEQ[0]}", ins=[], outs=[])
                    d.engine = eng
                    d.sync_info = mybir.SyncInfo(on_wait=[w], on_update=[])
                    out.append(d)
                ins.sync_info = mybir.SyncInfo(
                    on_wait=keep, on_update=list(si.on_update or [])
                )
                changed = True
                nsplit += 1
            out.append(ins)
        if changed:
            blk.instructions = out
    return nsplit


_NC_CACHE = None


def _get_nc():
    global _NC_CACHE
    if _NC_CACHE is None:
        nc = _build_nc()
        _split_waits(nc)
        _NC_CACHE = nc
    return _NC_CACHE


class Runner:
    """Persistent PJRT executor for an SPMD bass module (axon path).

    Mirrors bass2jax.run_bass_via_pjrt's multi-core branch but keeps the
    jitted callable so repeated (timed) invocations don't recompile.
    """

    def __init__(self, nc, n_cores=8):
        import jax
        from jax.experimental.shard_map import shard_map
        from jax.sharding import Mesh, PartitionSpec
        from concourse import bass2jax

        bass2jax.install_neuronx_cc_hook()
        self.jax = jax
        self.nc = nc
        self.n = n_cores
        partition_name = (
            nc.partition_id_tensor.name if nc.partition_id_tensor else None
        )
        in_names, out_names, out_avals = [], [], []
        for alloc in nc.m.functions[0].allocations:
            if not isinstance(alloc, mybir.MemoryLocationSet):
                continue
            name = alloc.memorylocations[0].name
            if alloc.kind == "ExternalInput":
                if name != partition_name:
                    in_names.append(name)
            elif alloc.kind == "ExternalOutput":
                out_names.append(name)
                out_avals.append(
                    jax.core.ShapedArray(
                        tuple(alloc.tensor_shape), mybir.dt.np(alloc.dtype)
                    )
                )
        self.in_names = list(in_names)
        self.out_names = out_names
        self.out_avals = out_avals
        bind_in_names = list(in_names) + list(out_names)
        if partition_name is not None:
            bind_in_names.append(partition_name)
        bind_in_names = tuple(bind_in_names)
        n_params = len(in_names)
        n_outs = len(out_names)

        def _body(*args):
            operands = list(args)
            if partition_name is not None:
                operands.append(bass2jax.partition_id_tensor())
            outs = bass2jax._bass_exec_p.bind(
                *operands,
                out_avals=tuple(out_avals),
                in_names=bind_in_names,
                out_names=tuple(out_names),
                lowering_input_output_aliases=(),
                sim_require_finite=True,
                sim_require_nnan=True,
                nc=nc,
            )
            return tuple(outs)

        devices = jax.devices()[:n_cores]
        self.mesh = Mesh(np.asarray(devices), ("core",))
        self.spec = PartitionSpec("core")
        in_specs = (self.spec,) * (n_params + n_outs)
        out_specs = (self.spec,) * n_outs
        donate = tuple(range(n_params, n_params + n_outs))
        self.fn = jax.jit(
            shard_map(
                _body,
                mesh=self.mesh,
                in_specs=in_specs,
                out_specs=out_specs,
                check_rep=False,
            ),
            donate_argnums=donate,
            keep_unused=True,
        )
        sharding = jax.sharding.NamedSharding(self.mesh, self.spec)
        self.zeros_fn = jax.jit(
            lambda: tuple(
                self.jax.numpy.zeros((n_cores * a.shape[0], *a.shape[1:]), a.dtype)
                for a in out_avals
            ),
            out_shardings=(sharding,) * n_outs,
        )

    def put_inputs(self, in_maps):
        """in_maps: per-core dict name->np.ndarray. Returns device arrays."""
        jax = self.jax
        sharding = jax.sharding.NamedSharding(self.mesh, self.spec)
        arrs = []
        for name in self.in_names:
            cat = np.concatenate([np.asarray(m[name]) for m in in_maps], axis=0)
            arrs.append(jax.device_put(cat, sharding))
        jax.block_until_ready(arrs)
        return arrs

    def __call__(self, dev_inputs):
        zs = self.zeros_fn()
        self.jax.block_until_ready(zs)
        out = self.fn(*dev_inputs, *zs)
        self.jax.block_until_ready(out)
        return out

    def time_it(self, dev_inputs, reps=10):
        import time as _t

        ts = []
        for _ in range(reps):
            zs = self.zeros_fn()
            self.jax.block_until_ready(zs)
            t0 = _t.perf_counter()
            out = self.fn(*dev_inputs, *zs)
            self.jax.block_until_ready(out)
            ts.append(_t.perf_counter() - t0)
        return ts

    def to_numpy(self, out):
        n = self.n
        return [
            {
                name: np.asarray(out[i]).reshape(n, *self.out_avals[i].shape)[c]
                for i, name in enumerate(self.out_names)
            }
            for c in range(n)
        ]


_RUNNER = None


def _get_runner():
    global _RUNNER
    if _RUNNER is None:
        _RUNNER = Runner(_get_nc(), B)
    return _RUNNER


def _prep_wb(w, b):
    # ws[p, g*9 + kh*3 + kw] = w[2g + p//64, kh, kw, p%64]
    w = np.asarray(w, dtype=np.float32).reshape(G, 2, 9, C)  # (g, dp, tap, c)
    ws = np.ascontiguousarray(w.transpose(1, 3, 0, 2).reshape(128, G * 9))
    b = np.asarray(b, dtype=np.float32).reshape(G, 2, C)
    bs = np.ascontiguousarray(b.transpose(1, 2, 0).reshape(128, G))
    return ws, bs


def _in_maps(inputs):
    x = np.asarray(inputs["x"], dtype=np.float32)
    ws, bs = _prep_wb(inputs["w"], inputs["b"])
    wbs = np.ascontiguousarray(np.concatenate([ws, bs], axis=1))
    return [{"xs": np.ascontiguousarray(x[i]), "wbs": wbs} for i in range(B)]


def kernel(**inputs) -> np.ndarray:
    r = _get_runner()
    dev = r.put_inputs(_in_maps(inputs))
    res = r.to_numpy(r(dev))
    return np.stack([m["ys"] for m in res], axis=0)


# revision 47
# speedup vs baseline: 2.7001x; 1.0078x over previous
"""Depthwise 3x3 conv over each depth slice of x[B,H,W,D,C] on 8 trn2 cores.

Strategy (v3 — engine-balanced range split, software-pipelined):
  - Data-parallel over batch: core i handles x[i] ([H,W,D,C] = [64,64,32,64]).
  - Per core, loop over 16 depth-pair groups; partitions = (d_parity, C) = 128,
    free axis = spatial (H*W) so per-(d,c) tap weights are per-partition
    scalars.
  - HBM has C contiguous, so the (spatial, channel) <-> (channel, spatial)
    layout change is done on-chip with PE transposes (128x128 blocks, f32r
    in / bf16 out at 1.5 / 1 cycles per row).
  - SAME zero padding handled by a 65-stride padded slab with zeroed guard
    rows/pad column so every tap is a flat shifted read.
  - The 64 spatial rows of each group are split across three engines so all
    of PE / DVE / ACT / Pool run ~13.5us per group:
      rows [0, PE_ROWS):            9 diag-matmuls (f32r, 1 cyc/row)
                                    accumulating in PSUM; ACT copies out,
                                    folding the bias via Identity-activation.
      rows [PE_ROWS, +DVE_ROWS):    tensor_scalar head (w*x+b, 2x mode)
                                    then 8 fused scalar_tensor_tensor taps,
                                    all on DVE.
      rows [.., 64):                head + 8 taps as TS-mult + TT-add pairs
                                    on GPSIMD (walrus rejects STT on Pool).
  - Diag matrices are built per group in ONE broadcast tensor_tensor on
    Pool; the conv result y2 is bf16 (halves out-transpose cost; ~2e-3
    rounding vs the 2e-2 gate; the psum->yd copy upcasts back to f32).
  - Phase pipeline per iteration p: DMA(p) + transposes/copies(p) are
    emitted one iteration ahead of the tap chains / PE conv (p-1), whose
    out-path trails one more group (p-2): PE's in-order stream
    [in-tr(p) | out-tr(p-2) | conv-mm(p-1)] then runs with zero stalls.
  - Sem-wait caps (PE Matmult takes 1 wait) are handled entirely by the
    _split_waits post-pass, which hoists excess waits onto same-engine
    Drain instructions.
"""

import os
from contextlib import ExitStack

import numpy as np

import concourse.bass as bass
import concourse.mybir as mybir
import concourse.tile as tile
from concourse.bass_utils import run_bass_kernel_spmd
from concourse.masks import make_identity
from concourse.tile import add_dep_helper

F32 = mybir.dt.float32
F32R = mybir.dt.float32r
BF16 = mybir.dt.bfloat16

B, H, W, D, C = 8, 64, 64, 32, 64
G = D // 2              # 16 depth-pair groups per core
RS = W + 1              # 65: padded row stride (col 64 of each row is zero)
DATA0 = RS + 1          # 66: flat offset of (h=0, w=0) in the slab
SLAB = DATA0 + 64 * RS + RS + 1   # 66 + 4160 + 66 = 4292
CONVL = 64 * RS         # 4160 = span of a [64 rows x 65] view

MULT = mybir.AluOpType.mult
ADD = mybir.AluOpType.add
IDENT_F = mybir.ActivationFunctionType.Identity

ALL_TAPS = [(dh, dw) for dh in (-1, 0, 1) for dw in (-1, 0, 1)]

# ---- tunables -----------------------------------------------------------
PE_ROWS = 37            # rows on the TensorEngine (diag-matmul taps)
DVE_ROWS = 22           # rows on the vector engine (STT chain)
POOL_ROWS = 64 - PE_ROWS - DVE_ROWS
POOL_STT = False        # walrus rejects STT on Pool -> TS-mult + TT-add
SLAB_DT = F32R          # slab/diag dtype fed to PE matmuls (1 cyc/row)
TRANS_F32R = True       # run the 128x128 transposes as f32r (1.5 vs 2 cyc)
Y2_DT = BF16            # conv result dtype (bf16 = 1cyc/row out-transposes)
ABSORBERS = True        # PE observer matmuls (cheap; shields 1-wait cap)
DIAG_ON_POOL = True     # diag built on gpsimd (DVE fallback)
TRANS_LAG = 0           # iterations between dma issue and transposes
CONV_LAG = 1            # iterations between dma issue and conv
OUT_OFF = 1             # groups between conv and its out-path
PIN_W = 4               # transposes per in-psum tile (4 or 8)
XD_BUFS = 3
XA_BUFS = 3
SC_BUFS = 2
Y2_BUFS = 2
YD_BUFS = 2
PIN_BUFS = 3
PCV_BUFS = 2
POUT_BUFS = 2
POUT_W = 1              # 128-blocks per out-psum tile / 4
LAST_ROWS = (37, 22, 5)  # row split for the final group (drain latency)
FIRST_ROWS = (37, 22, 5)  # row split for group 0 (ramp latency)
LAST2_ROWS = (37, 22, 5)  # row split for group G-2
OUT_POS = 'mid'          # out-path emission: early/mid/late in the iteration
OUT_DMA_SPLIT = 8        # stores per group (stream during out-copies)
DVE_HEAD_ACT = False     # compute the DVE-range head on ACT instead


def _rows_for(g):
    """(pe, dve, pool) rows for group g; the last group biases toward PE
    to shorten its serial DVE chain (it is the pipeline drain)."""
    if g == G - 1:
        return LAST_ROWS
    if g == G - 2:
        return LAST2_ROWS
    if g == 0:
        return FIRST_ROWS
    return (PE_ROWS, DVE_ROWS, POOL_ROWS)


def _pe_chunks(pe_rows):
    """Split pe_rows into psum-sized chunks (>=4 rows keeps f32r fast)."""
    out, r = [], 0
    while r < pe_rows:
        n = min(8, pe_rows - r)
        out.append((r, n))
        r += n
    assert all(n >= 4 for _, n in out)
    return out


def _build_nc():
    nc = bass.Bass("TRN2", target_bir_lowering=False, debug=False)
    xs = nc.dram_tensor("xs", [H, W, D, C], F32, kind="ExternalInput").ap()
    wbs = nc.dram_tensor("wbs", [128, G * 9 + G], F32, kind="ExternalInput").ap()
    ys = nc.dram_tensor("ys", [H, W, D, C], F32, kind="ExternalOutput").ap()

    with tile.TileContext(nc) as tc, ExitStack() as ctx:
        consts = ctx.enter_context(tc.tile_pool(name="consts", bufs=1))
        ident = consts.tile([128, 128], F32)
        make_identity(nc, ident[:])
        # f32r identity for the transposes: walrus wants every compute
        # producer feeding an f32r matmul to declare an f32r-rounded output,
        # so materialize it via an ACT copy (f32r out is legal on ACT).
        identr = consts.tile([128, 128], F32R)
        nc.scalar.copy(identr[:], ident[:])
        identb16 = consts.tile([128, 128], BF16)
        nc.scalar.copy(identb16[:], ident[:])
        wbst = consts.tile([128, G * 9 + G], F32)
        nc.sync.dma_start(wbst[:], wbs)
        wst = wbst[:, 0 : G * 9]
        bst = wbst[:, G * 9 : G * 9 + G]

        xdp = ctx.enter_context(tc.tile_pool(name="xd", bufs=XD_BUFS))
        xap = ctx.enter_context(tc.tile_pool(name="xa", bufs=XA_BUFS))
        dgp = ctx.enter_context(tc.tile_pool(name="diag", bufs=3))
        scp = ctx.enter_context(tc.tile_pool(name="scr", bufs=SC_BUFS))
        y2p = ctx.enter_context(tc.tile_pool(name="y2", bufs=Y2_BUFS))
        ydp = ctx.enter_context(tc.tile_pool(name="yd", bufs=YD_BUFS))
        pin = ctx.enter_context(
            tc.tile_pool(name="pin", bufs=PIN_BUFS, space=bass.MemorySpace.PSUM)
        )
        pcv = ctx.enter_context(
            tc.tile_pool(name="pcv", bufs=PCV_BUFS, space=bass.MemorySpace.PSUM)
        )
        pout = ctx.enter_context(
            tc.tile_pool(name="pout", bufs=POUT_BUFS, space=bass.MemorySpace.PSUM)
        )
        dummy = None
        if ABSORBERS:
            pdum = ctx.enter_context(
                tc.tile_pool(name="pdum", bufs=1, space=bass.MemorySpace.PSUM)
            )
            dummy = pdum.tile([128, 8], F32)

        def pe_absorb(col, dep=None):
            if not ABSORBERS:
                return None
            if col.dtype is not F32:
                col = col.bitcast(F32)
            mm = nc.tensor.matmul(
                dummy[0:1, 0:1], col, ident[:, 0:1], skip_group_check=True
            )
            if dep is not None:
                add_dep_helper(mm.ins, dep.ins, reason="observe")
            return mm

        def pin_dep(mm, *deps):
            for d in deps:
                if d is not None:
                    add_dep_helper(mm.ins, d.ins, reason="after-observer")

        pe_absorb(ident[:, 0:1])  # PE observes the identity build once

        # ---- software pipeline: in-path runs TWO groups ahead of the
        # conv so the PE matmuls never wait on the slab handoff (ACT
        # copies) inside PE's in-order stream, and the tap chains start
        # each cycle with their slab already resident.
        in_state = {}
        conv_state = {}

        def dma_issue(g):
            src = xs[:, :, 2 * g : 2 * g + 2, :].rearrange(
                "(j ph) w dp c -> (ph w) j (dp c)", ph=2
            )
            # xd is f32r-typed so the DMA itself is the declared f32r
            # producer for the f32r in-transposes (same bits as f32)
            xd = xdp.tile([128, 32, 128], F32R if TRANS_F32R else F32, tag="xd")
            sv = src if not TRANS_F32R else src.bitcast(F32R)
            if g == 0:
                nc.sync.dma_start(xd[:, 0:8, :], sv[:, 0:8, :])
                nc.sync.dma_start(xd[:, 8:32, :], sv[:, 8:32, :])
            else:
                nc.sync.dma_start(xd[:], sv)
            in_state[g] = dict(xd=xd)

        def trans_copy(g):
            xd = in_state[g]["xd"]
            xa = xap.tile([128, SLAB], SLAB_DT, tag="xa")
            pad_last = None
            if g < XA_BUFS:
                # the guard rows / pad column are disjoint from the data
                # copies (which write cols 0:64 of rows 0:64 only), so they
                # persist across buffer reuse: zero each physical buffer once
                xaf = xa[:] if SLAB_DT is F32 else xa[:].bitcast(F32)
                nc.gpsimd.memset(xaf[:, 0:DATA0], 0.0)
                nc.gpsimd.memset(xaf[:, DATA0 + 63 * RS + 64 : SLAB], 0.0)
                padcol = xaf[:, DATA0 + 64 : DATA0 + 64 + CONVL].rearrange(
                    "p (r o) -> p r o", o=RS
                )[:, :, 0:1]
                pad_last = nc.gpsimd.memset(padcol, 0.0)

            absA = pe_absorb(xd[:, 0, 0:1])  # PE observes xd's DMA
            last_copy = None
            # tail (DVE/Pool range) chunks first so their chains can start
            nin = 32 // PIN_W
            order = list(range(nin // 2, nin)) + list(range(nin // 2))
            for k in order:
                pt = pin.tile([128, 128 * PIN_W], F32, tag="pin")
                for jo in range(PIN_W):
                    j = PIN_W * k + jo
                    if TRANS_F32R:
                        t = nc.tensor.transpose(
                            pt[:, 128 * jo : 128 * (jo + 1)].bitcast(F32R),
                            xd[:, j, :],
                            identr[:],
                        )
                    else:
                        t = nc.tensor.transpose(
                            pt[:, 128 * jo : 128 * (jo + 1)], xd[:, j, :], ident[:]
                        )
                    pin_dep(t, absA)
                span = 130 * PIN_W
                dst = xa[:, DATA0 + span * k : DATA0 + span * k + span].rearrange(
                    "p (j r b) -> p j r b", j=PIN_W, b=RS
                )[:, :, :, 0:64]
                srcp = pt[:].rearrange("p (j r b) -> p j r b", j=PIN_W, b=64)
                last_copy = nc.scalar.copy(dst, srcp)

            # diag for the PE taps: ONE broadcast tensor_tensor (on Pool,
            # which has slack; f32r out keeps walrus's producer rule happy)
            diag = dgp.tile([128, 9 * 128], SLAB_DT, tag="diag")
            identb = ident[:].unsqueeze(1).broadcast_to([128, 9, 128])
            wb = (
                wst[:, g * 9 : g * 9 + 9]
                .unsqueeze(2)
                .broadcast_to([128, 9, 128])
            )
            dgv = diag[:].rearrange("p (t c) -> p t c", t=9)
            if DIAG_ON_POOL:
                diag_done = nc.gpsimd.tensor_tensor(dgv, identb, wb, MULT)
            else:
                diag_done = nc.vector.tensor_tensor(dgv, identb, wb, MULT)

            in_state[g].update(
                xa=xa, diag=diag, last_copy=last_copy, pad_last=pad_last,
                diag_done=diag_done,
            )

        def xsh(g, dh, dw, r0, nr, dt=True):
            xa = in_state[g]["xa"]
            s0 = DATA0 + dh * RS + dw + r0 * RS
            v = xa[:, s0 : s0 + nr * RS]
            if dt and SLAB_DT is not F32:
                v = v.bitcast(F32)
            return v.rearrange("p (r b) -> p r b", b=RS)[:, :, 0:64]

        def wap(g, dh, dw):
            i = g * 9 + (dh + 1) * 3 + (dw + 1)
            return wst[:, i : i + 1]

        def chains(g):
            bias = bst[:, g : g + 1]
            y2 = y2p.tile([128, 4096], Y2_DT, tag="y2")
            conv_state[g] = dict(y2=y2)

            def yv(r0, nr):
                return y2[:, r0 * 64 : (r0 + nr) * 64].rearrange(
                    "p (r w) -> p r w", w=64
                )

            pe_r, dve_r, pool_r = _rows_for(g)
            r_dve = pe_r
            r_pool = pe_r + dve_r
            (h0, w0), rest = ALL_TAPS[0], ALL_TAPS[1:]
            sc_d = scp.tile([128, DVE_ROWS * 64], F32, tag="scd")
            sc_dv = sc_d[:, 0 : dve_r * 64].rearrange("p (r w) -> p r w", w=64)
            if DVE_HEAD_ACT:
                nc.scalar.activation(
                    sc_dv, xsh(g, h0, w0, r_dve, dve_r), IDENT_F,
                    bias=bias, scale=wap(g, h0, w0),
                )
            else:
                nc.vector.tensor_scalar(
                    sc_dv, xsh(g, h0, w0, r_dve, dve_r), wap(g, h0, w0), bias,
                    MULT, ADD,
                )
            dve_last = None
            for i, (dh, dw) in enumerate(rest):
                out = yv(r_dve, dve_r) if i == len(rest) - 1 else sc_dv
                dve_last = nc.vector.scalar_tensor_tensor(
                    out, xsh(g, dh, dw, r_dve, dve_r), wap(g, dh, dw),
                    sc_dv, MULT, ADD,
                )

            pool_last = None
            if pool_r:
                sc_p = scp.tile([128, max(POOL_ROWS, LAST_ROWS[2]) * 64], F32,
                                tag="scp")
                sc_pv = sc_p[:, 0 : pool_r * 64].rearrange(
                    "p (r w) -> p r w", w=64
                )
                nc.gpsimd.tensor_scalar(
                    sc_pv, xsh(g, h0, w0, r_pool, pool_r), wap(g, h0, w0),
                    bias, MULT, ADD,
                )
                sc_q = scp.tile([128, max(POOL_ROWS, LAST_ROWS[2]) * 64], F32,
                                tag="scq")
                sc_qv = sc_q[:, 0 : pool_r * 64].rearrange(
                    "p (r w) -> p r w", w=64
                )
                for i, (dh, dw) in enumerate(rest):
                    out = yv(r_pool, pool_r) if i == len(rest) - 1 else sc_pv
                    nc.gpsimd.tensor_scalar(
                        sc_qv, xsh(g, dh, dw, r_pool, pool_r),
                        wap(g, dh, dw), None, MULT,
                    )
                    pool_last = nc.gpsimd.tensor_tensor(out, sc_pv, sc_qv, ADD)
            conv_state[g]["dve_last"] = dve_last
            conv_state[g]["pool_last"] = pool_last

        def pe_conv(g):
            st = in_state[g]
            bias = bst[:, g : g + 1]
            y2 = conv_state[g]["y2"]
            diag = st["diag"]
            abs_xa = pe_absorb(ident[:, 0:1], dep=st["last_copy"])
            abs_pd = pe_absorb(ident[:, 0:1], dep=st["pad_last"])
            abs_dg = pe_absorb(ident[:, 0:1], dep=st["diag_done"])
            pe_copy_last = None
            for r0, nr in _pe_chunks(_rows_for(g)[0]):
                Pq = pcv.tile([128, nr * 64], F32, tag="pcv")
                for i, (dh, dw) in enumerate(ALL_TAPS):
                    if SLAB_DT is F32:
                        lhs = diag[:, 128 * i : 128 * (i + 1)].bitcast(F32R)
                        rhs = xsh(g, dh, dw, r0, nr, dt=False).bitcast(F32R)
                    else:
                        lhs = diag[:, 128 * i : 128 * (i + 1)]
                        rhs = xsh(g, dh, dw, r0, nr, dt=False)
                    mm = nc.tensor.matmul(
                        Pq[:], lhs, rhs,
                        start=(i == 0), stop=(i == len(ALL_TAPS) - 1),
                    )
                    pin_dep(mm, abs_xa, abs_pd, abs_dg)
                pe_copy_last = nc.scalar.activation(
                    y2[:, r0 * 64 : (r0 + nr) * 64].rearrange(
                        "p (r w) -> p r w", w=64
                    ),
                    Pq[:].rearrange("p (r w) -> p r w", w=64),
                    IDENT_F, bias=bias,
                )
            conv_state[g]["pe_copy_last"] = pe_copy_last

        def out_path(g):
            st = conv_state.pop(g)
            y2 = st["y2"]
            yd = ydp.tile([128, 32, 128], F32, tag="yd")
            a1 = pe_absorb(ident[:, 0:1], dep=st["dve_last"])
            a2 = pe_absorb(ident[:, 0:1], dep=st["pool_last"])
            a3 = pe_absorb(ident[:, 0:1], dep=st["pe_copy_last"])
            for q in range(8 // POUT_W):
                pt = pout.tile([128, 512 * POUT_W],
                               Y2_DT if Y2_DT is BF16 else F32, tag="pout")
                for jo in range(4 * POUT_W):
                    j = 4 * POUT_W * q + jo
                    yblk = y2[:, 128 * j : 128 * (j + 1)]
                    if Y2_DT is F32R:
                        t = nc.tensor.transpose(
                            pt[:, 128 * jo : 128 * (jo + 1)].bitcast(F32R),
                            yblk, identr[:],
                        )
                    elif Y2_DT is BF16:
                        t = nc.tensor.transpose(
                            pt[:, 128 * jo : 128 * (jo + 1)], yblk, identb16[:]
                        )
                    else:
                        t = nc.tensor.transpose(
                            pt[:, 128 * jo : 128 * (jo + 1)], yblk, ident[:]
                        )
                    pin_dep(t, a1, a2, a3)
                nc.scalar.copy(
                    yd[:, 4 * POUT_W * q : 4 * POUT_W * (q + 1), :],
                    pt[:].rearrange("p (j c) -> p j c", j=4 * POUT_W),
                )
            dst = ys[:, :, 2 * g : 2 * g + 2, :].rearrange(
                "(j ph) w dp c -> (ph w) j (dp c)", ph=2
            )
            for k in range(OUT_DMA_SPLIT):
                n = 32 // OUT_DMA_SPLIT
                nc.sync.dma_start(
                    dst[:, n * k : n * (k + 1), :], yd[:, n * k : n * (k + 1), :]
                )

        for p in range(G + CONV_LAG + 1):
            if p < G:
                dma_issue(p)
            if TRANS_LAG <= p < G + TRANS_LAG:
                trans_copy(p - TRANS_LAG)
            if CONV_LAG <= p < G + CONV_LAG:
                g = p - CONV_LAG
                if OUT_POS == 'early' and g >= OUT_OFF:
                    out_path(g - OUT_OFF)
                chains(g)
                if OUT_POS == 'mid' and g >= OUT_OFF:
                    out_path(g - OUT_OFF)
                pe_conv(g)
                if OUT_POS == 'late' and g >= OUT_OFF:
                    out_path(g - OUT_OFF)
        for g in range(G - OUT_OFF, G):
            out_path(g)

    return nc


# walrus setupSyncWait caps per engine struct: PE Matmult takes 1 sem wait,
# ACT/DVE/Pool compute ops take 2. Tile sometimes attaches more (psum slot
# release-sets). Hoist the excess onto injected same-engine Drains (Tile's
# own epilogue Drain carries 12 waits, so Drain accepts many).
_WAIT_CAPS = {"PE": 1, "Activation": 1, "DVE": 1, "Pool": 1, "SP": 1}
_SPLIT_SEQ = [0]


def _split_waits(nc):
    fn = nc.m.functions[0]
    nsplit = 0
    for blk in fn.blocks:
        out = []
        changed = False
        for ins in blk.instructions:
            si = ins.sync_info
            waits = list(si.on_wait) if si is not None and si.on_wait else []
            eng = getattr(ins, "engine", None)
            engname = getattr(eng, "value", None) or str(eng)
            cap = _WAIT_CAPS.get(engname)
            if cap is not None and len(waits) > cap:
                excess, keep = waits[:-cap], waits[-cap:]
                for w in excess:
                    _SPLIT_SEQ[0] += 1
                    d = mybir.InstDrain(name=f"I-ws{_SPLIT_SEQ[0]}", ins=[], outs=[])
                    d.engine = eng
                    d.sync_info = mybir.SyncInfo(on_wait=[w], on_update=[])
                    out.append(d)
                ins.sync_info = mybir.SyncInfo(
                    on_wait=keep, on_update=list(si.on_update or [])
                )
                changed = True
                nsplit += 1
            out.append(ins)
        if changed:
            blk.instructions = out
    return nsplit


_NC_CACHE = None


def _get_nc():
    global _NC_CACHE
    if _NC_CACHE is None:
        nc = _build_nc()
        _split_waits(nc)
        _NC_CACHE = nc
    return _NC_CACHE


class Runner:
    """Persistent PJRT executor for an SPMD bass module (axon path).

    Mirrors bass2jax.run_bass_via_pjrt's multi-core branch but keeps the
    jitted callable so repeated (timed) invocations don't recompile.
    """

    def __init__(self, nc, n_cores=8):
        import jax
        from jax.experimental.shard_map import shard_map
        from jax.sharding import Mesh, PartitionSpec
        from concourse import bass2jax

        bass2jax.install_neuronx_cc_hook()
        self.jax = jax
        self.nc = nc
        self.n = n_cores
        partition_name = (
            nc.partition_id_tensor.name if nc.partition_id_tensor else None
        )
        in_names, out_names, out_avals = [], [], []
        for alloc in nc.m.functions[0].allocations:
            if not isinstance(alloc, mybir.MemoryLocationSet):
                continue
            name = alloc.memorylocations[0].name
            if alloc.kind == "ExternalInput":
                if name != partition_name:
                    in_names.append(name)
            elif alloc.kind == "ExternalOutput":
                out_names.append(name)
                out_avals.append(
                    jax.core.ShapedArray(
                        tuple(alloc.tensor_shape), mybir.dt.np(alloc.dtype)
                    )
                )
        self.in_names = list(in_names)
        self.out_names = out_names
        self.out_avals = out_avals
        bind_in_names = list(in_names) + list(out_names)
        if partition_name is not None:
            bind_in_names.append(partition_name)
        bind_in_names = tuple(bind_in_names)
        n_params = len(in_names)
        n_outs = len(out_names)

        def _body(*args):
            operands = list(args)
            if partition_name is not None:
                operands.append(bass2jax.partition_id_tensor())
            outs = bass2jax._bass_exec_p.bind(
                *operands,
                out_avals=tuple(out_avals),
                in_names=bind_in_names,
                out_names=tuple(out_names),
                lowering_input_output_aliases=(),
                sim_require_finite=True,
                sim_require_nnan=True,
                nc=nc,
            )
            return tuple(outs)

        devices = jax.devices()[:n_cores]
        self.mesh = Mesh(np.asarray(devices), ("core",))
        self.spec = PartitionSpec("core")
        in_specs = (self.spec,) * (n_params + n_outs)
        out_specs = (self.spec,) * n_outs
        donate = tuple(range(n_params, n_params + n_outs))
        self.fn = jax.jit(
            shard_map(
                _body,
                mesh=self.mesh,
                in_specs=in_specs,
                out_specs=out_specs,
                check_rep=False,
            ),
            donate_argnums=donate,
            keep_unused=True,
        )
        sharding = jax.sharding.NamedSharding(self.mesh, self.spec)
        self.zeros_fn = jax.jit(
            lambda: tuple(
                self.jax.numpy.zeros((n_cores * a.shape[0], *a.shape[1:]), a.dtype)
                for a in out_avals
            ),
            out_shardings=(sharding,) * n_outs,
        )

    def put_inputs(self, in_maps):
        """in_maps: per-core dict name->np.ndarray. Returns device arrays."""
        jax = self.jax
        sharding = jax.sharding.NamedSharding(self.mesh, self.spec)
        arrs = []
        for name in self.in_names:
            cat = np.concatenate([np.asarray(m[name]) for m in in_maps], axis=0)
            arrs.append(jax.device_put(cat, sharding))
        jax.block_until_ready(arrs)
        return arrs

    def __call__(self, dev_inputs):
        zs = self.zeros_fn()
        self.jax.block_until_ready(zs)
        out = self.fn(*dev_inputs, *zs)
        self.jax.block_until_ready(out)
        return out

    def time_it(self, dev_inputs, reps=10):
        import time as _t

        ts = []
        for _ in range(reps):
            zs = self.zeros_fn()
            self.jax.block_until_ready(zs)
            t0 = _t.perf_counter()
            out = self.fn(*dev_inputs, *zs)
            self.jax.block_until_ready(out)
            ts.append(_t.perf_counter() - t0)
        return ts

    def to_numpy(self, out):
        n = self.n
        return [
            {
                name: np.asarray(out[i]).reshape(n, *self.out_avals[i].shape)[c]
                for i, name in enumerate(self.out_names)
            }
            for c in range(n)
        ]


_RUNNER = None


def _get_runner():
    global _RUNNER
    if _RUNNER is None:
        _RUNNER = Runner(_get_nc(), B)
    return _RUNNER


def _prep_wb(w, b):
    # ws[p, g*9 + kh*3 + kw] = w[2g + p//64, kh, kw, p%64]
    w = np.asarray(w, dtype=np.float32).reshape(G, 2, 9, C)  # (g, dp, tap, c)
    ws = np.ascontiguousarray(w.transpose(1, 3, 0, 2).reshape(128, G * 9))
    b = np.asarray(b, dtype=np.float32).reshape(G, 2, C)
    bs = np.ascontiguousarray(b.transpose(1, 2, 0).reshape(128, G))
    return ws, bs


def _in_maps(inputs):
    x = np.asarray(inputs["x"], dtype=np.float32)
    ws, bs = _prep_wb(inputs["w"], inputs["b"])
    wbs = np.ascontiguousarray(np.concatenate([ws, bs], axis=1))
    return [{"xs": np.ascontiguousarray(x[i]), "wbs": wbs} for i in range(B)]


def kernel(**inputs) -> np.ndarray:
    r = _get_runner()
    dev = r.put_inputs(_in_maps(inputs))
    res = r.to_numpy(r(dev))
    return np.stack([m["ys"] for m in res], axis=0)
